# revision 25
# baseline (speedup 1.0000x reference)
"""Multi-head attention (B=2, S=2048, D=1024, H=16, causal mask) on 8 trn2
NeuronCores.

Sharding: 2-way data parallel over batch x 4-way tensor parallel over head
groups (4 heads / core).  Core c handles batch c//4, head group c%4.

Everything on-chip lives feature-major ("transposed") so no transposes are
ever needed: inputs arrive chunk-major [P, chunk, ftile, t] in fp16; Q/K
projections produce Qh^T/Kh^T [e, t]; scores come out keys-major [k, q];
exp(p) feeds A@V directly as the moving operand with V (+ a ones column
that makes the softmax denominator fall out of the same matmul)
stationary; the attention output appears as x_att^T [e, q], which is
exactly the layout the output projection wants.  Each core emits its y^T
partial and the host sums the 4 partials of each batch group (row-parallel
tensor-parallel reduction).

Differences vs the phase-separated v1:
  * single software-pipelined loop over query chunks: the projection
    matmuls of chunk c+1 fill the tensor-engine gaps left while ScalarE
    runs the exp stream of chunk c (the Tile scheduler interleaves by
    priority/readiness; ScalarE exp is the critical resource at ~92us).
  * K bias dropped entirely (adding bk shifts every score column by a
    per-query constant, which softmax cancels exactly).
  * V bias and output bias are applied analytically on the host
    (bv @ wo.T + bo added once after the partial-sum gather).
  * softmax denominator reciprocal as a batched 2-row Ln + Exp(-x) pair
    on ScalarE per head pair; the recip rows partition-broadcast through
    a DRAM bounce mid-kernel, and via a K=1 ones matmul on the (otherwise
    idle) PE for the final pair so the last output projection is not
    delayed and stays on a warm clock.
  * exp always runs full-width [128, 2*QC] (one ACTIVATE per k-tile per
    head pair); scores matmuls write the full query range so the psum is
    fully initialized.  Mask patterns zero the dead blocks after exp.
  * junk matmuls warm the PE HAM clock gate while the first input chunk
    streams in (and again under the final normalize chain).
  * y partials stream out as fp16 directly from a psum->sbuf cast.

Mask handled generically: the [S,S] mask is classified on the host into
128x128 blocks (zero / one / mixed).  Zero k-tiles are skipped entirely,
mixed blocks get a pattern-multiply after exp with deduplicated patterns
uploaded as data.
"""

import os
import sys

import numpy as np

for _p in ("/opt/trn_rl_repo", "/root/.axon_site/_ro/trn_rl_repo"):
    if os.path.isdir(_p) and _p not in sys.path:
        sys.path.append(_p)

import ml_dtypes  # noqa: E402
from contextlib import ExitStack  # noqa: E402

import concourse.bass as bass  # noqa: E402
import concourse.tile as tile  # noqa: E402
from concourse import mybir  # noqa: E402

# ----- problem constants (hardcoded per contract) ---------------------------
B, S, D, H, DK = 2, 2048, 1024, 16, 64
NCORES = 8
TP = 4                      # head-parallel ways (per batch group)
EL = D // TP                # 256 local head dims = 4 heads
HL = H // TP                # 4 local heads
QC = 512                    # query-chunk (columns per attention pass)
NQC = S // QC               # 4
KT = 128                    # key tile (contraction tile for A@V)
NKT = S // KT               # 16
P = 128
NMT = D // P                # 8 output-feature tiles
SCALE = 1.0 / np.sqrt(DK)

F32 = mybir.dt.float32
F16 = mybir.dt.float16
F16NP = np.float16


# ----- host-side mask analysis ---------------------------------------------
class _KTile:
    __slots__ = ("kt", "s0", "s1", "muls", "first", "last")

    def __init__(self, kt, s0, s1, muls):
        self.kt, self.s0, self.s1, self.muls = kt, s0, s1, muls
        self.first = False
        self.last = False


def _mask_plan(mask2d):
    """mask2d: [S, S] ints, mask2d[q, k] (1 = attend).  Returns
    (plan, patterns) where plan[qc] is a list of _KTile and patterns is a
    f16 array [n_pat, 128, 128] of transposed (k-major) mask blocks."""
    mT = (mask2d != 0).astype(np.float32).T          # [k, q]
    nqt = S // KT
    blk = mT.reshape(NKT, KT, nqt, KT).transpose(0, 2, 1, 3)  # [kt, qt, 128, 128]
    sums = blk.sum(axis=(2, 3))
    patterns = []
    pat_idx = {}

    def pattern_id(kt, qt):
        key = blk[kt, qt].tobytes()
        if key not in pat_idx:
            pat_idx[key] = len(patterns)
            patterns.append(blk[kt, qt].astype(np.float16))
        return pat_idx[key]

    qt_per_qc = QC // KT
    plan = []
    for qc in range(NQC):
        tiles = []
        for kt in range(NKT):
            sub = sums[kt, qc * qt_per_qc:(qc + 1) * qt_per_qc]
            nz = [i for i in range(qt_per_qc) if sub[i] > 0]
            if not nz:
                continue
            s0, s1 = nz[0] * KT, (nz[-1] + 1) * KT
            tiles.append(_KTile(kt, s0, s1, None))
        if not tiles:
            raise ValueError(f"query chunk {qc} has no unmasked keys")
        u0 = min(t.s0 for t in tiles)
        u1 = max(t.s1 for t in tiles)
        tiles[0].s0, tiles[0].s1 = u0, u1
        tiles[0].first = True
        tiles[-1].last = True
        for t in tiles:
            muls = []
            for qt in range(t.s0 // KT, t.s1 // KT):
                full = sums[t.kt, qc * qt_per_qc + qt]
                if full != KT * KT:          # zero or mixed -> needs pattern
                    muls.append((qt, pattern_id(t.kt, qc * qt_per_qc + qt)))
            t.muls = muls
        plan.append(tiles)
    pats = np.stack(patterns) if patterns else np.zeros((1, KT, KT), np.float16)
    return plan, pats


# ----- TileContext with a codegen-safe exit drain ---------------------------
# The stock kernel-tail drain carries one semaphore wait per engine/queue the
# kernel touched; CoreV3 codegen rejects instructions with more than two
# waits ("Too many sync wait commands").  Split the waits across preceding
# sync-engine nops, two per instruction, so the drain itself needs none.
class _TileContext(tile.TileContext):
    def _drain_and_barrier(self, tick_clock, wait_clock):
        from concourse.vector_clock import ScopedClock
        nc = self.nc
        probe = nc.sync.nop()
        wait_clock.add_sem_waits(
            probe.ins, ScopedClock({None: tick_clock.global_clock}))
        si = probe.ins.sync_info
        waits = list(si.on_wait) if si and si.on_wait else []
        if len(waits) > 1:
            probe.ins.sync_info = mybir.SyncInfo(
                on_wait=waits[:1], on_update=list(si.on_update or []))
            for w in waits[1:]:
                n = nc.sync.nop()
                n.ins.sync_info = mybir.SyncInfo(on_wait=[w], on_update=[])
        nc.sync.drain()
        nc.all_engine_barrier()
        assert self.sems is not None
        popped = nc._tile_sem_poison_stack.pop()
        assert popped is self._sem_poison
        nc.clear_and_free_semaphores(list(self.sems.allocated().values()))
        nc.all_engine_barrier()


# The same wait-count limit applies to ordinary engine instructions under
# this walrus build, so after the program is fully built, hoist all but one
# wait of every instruction onto preceding same-engine no-ops.
def _legalize_waits(nc, limit=1):
    for bb in nc.main_func.blocks:
        insts = list(bb.instructions)
        out = []
        for inst in insts:
            si = inst.sync_info
            waits = list(si.on_wait) if si and si.on_wait else []
            if len(waits) > limit:
                for w in waits[:-limit]:
                    nop = mybir.InstNoOp(
                        name=nc.get_next_instruction_name(), ins=[], outs=[])
                    nop.engine = inst.engine
                    nop.sync_info = mybir.SyncInfo(on_wait=[w], on_update=[])
                    nc.register_instruction(nop, overwrite=True)
                    out.append(nop)
                inst.sync_info = mybir.SyncInfo(
                    on_wait=waits[-limit:],
                    on_update=list(si.on_update or []))
            out.append(inst)
        bb.instructions = out


# ----- the bass program -----------------------------------------------------
def build_program(plan, n_pat):
    nc = bass.Bass(num_devices=NCORES)

    xq2 = nc.dram_tensor("xq2", [P, NQC * 8 * QC], F16, kind="ExternalInput")
    xk2 = nc.dram_tensor("xk2", [P, NQC * 8 * QC], F16, kind="ExternalInput")
    xv2 = nc.dram_tensor("xv2", [P, NQC * 8 * QC], F16, kind="ExternalInput")
    wqT = nc.dram_tensor("wqT", [D, EL], F16, kind="ExternalInput")
    wkT = nc.dram_tensor("wkT", [D, EL], F16, kind="ExternalInput")
    wvT = nc.dram_tensor("wvT", [D, EL], F16, kind="ExternalInput")
    woT = nc.dram_tensor("woT", [EL, D], F16, kind="ExternalInput")
    bq2 = nc.dram_tensor("bq2", [2, P], F32, kind="ExternalInput")
    pats = nc.dram_tensor("pats", [n_pat, KT, KT], F16, kind="ExternalInput")
    yO = nc.dram_tensor("yO", [NMT * NQC * P, QC], F16, kind="ExternalOutput")

    xrs = {
        "q": xq2.rearrange("p (c a t) -> p c a t", c=NQC, a=8),
        "k": xk2.rearrange("p (c a t) -> p c a t", c=NQC, a=8),
        "v": xv2.rearrange("p (c a t) -> p c a t", c=NQC, a=8),
    }

    with ExitStack() as ctx:
        tc = ctx.enter_context(_TileContext(nc))
        singles = ctx.enter_context(tc.tile_pool(name="singles", bufs=1))

        # --- persistent SBUF state ---
        wq_sb = singles.tile([P, 8, EL], F16)
        wk_sb = singles.tile([P, 8, EL], F16)
        wv_sb = singles.tile([P, 8, EL], F16)
        wo_sb = singles.tile([P, 2, D], F16)
        nc.sync.dma_start(out=wq_sb[:], in_=wqT.rearrange("(a p) e -> p a e", p=P))
        nc.sync.dma_start(out=wk_sb[:], in_=wkT.rearrange("(a p) e -> p a e", p=P))
        nc.sync.dma_start(out=wv_sb[:], in_=wvT.rearrange("(a p) e -> p a e", p=P))
        nc.sync.dma_start(out=wo_sb[:], in_=woT.rearrange("(a p) m -> p a m", p=P))
        bq_sb = singles.tile([P, 2], F32)
        nc.sync.dma_start(out=bq_sb[:], in_=bq2.rearrange("a p -> p a"))
        pat_sb = singles.tile([P, n_pat, KT], F16)
        nc.sync.dma_start(out=pat_sb[:], in_=pats.rearrange("n p k -> p n k"))

        Qt = singles.tile([P, 2, S], F16)     # [e-within-tile, e-tile, t]
        Kt = singles.tile([P, 2, S], F16)
        Vaug = singles.tile([P, NKT, HL, DK + 1], F16)  # [t-in-ktile, kt, h, e|1]
        nc.vector.memset(Vaug[:, :, :, DK:DK + 1], 1.0)
        ones_f32 = singles.tile([P, DK], F32)
        nc.vector.memset(ones_f32[:], 1.0)
        ones_col = singles.tile([P, 1], F16)
        nc.vector.memset(ones_col[:], 1.0)

        # warm the Exp activation table while the inputs stream in
        warms = singles.tile([1, 8], F32)
        nc.scalar.activation(out=warms[0:1, 0:1], in_=bq_sb[0:1, 0:1],
                             func=mybir.ActivationFunctionType.Exp)

        # --- pools ---
        xin = ctx.enter_context(tc.tile_pool(name="xin", bufs=2))
        spool = ctx.enter_context(tc.tile_pool(name="spool", bufs=2, space="PSUM"))
        avy = ctx.enter_context(tc.tile_pool(name="avy", bufs=2, space="PSUM"))
        gps = ctx.enter_context(tc.tile_pool(name="gps", bufs=2, space="PSUM"))
        ptp = ctx.enter_context(tc.tile_pool(name="ptp", bufs=NKT + 2))
        xtp = ctx.enter_context(tc.tile_pool(name="xtp", bufs=2))
        nrm = ctx.enter_context(tc.tile_pool(name="nrm", bufs=4))
        ysb = ctx.enter_context(tc.tile_pool(name="ysb", bufs=3))
        dbp = ctx.enter_context(tc.tile_pool(name="dbp", bufs=2, space="DRAM"))

        # --- PE HAM warm-up: junk matmuls while the first chunk loads ---
        wrm = singles.tile([P, QC], F16)
        nc.vector.memset(wrm[:], 0.0)
        wps = gps.tile([P, QC], F32, tag="gp", name="warmps")
        for _ in range(10):
            nc.tensor.matmul(wps[:], lhsT=wrm[:, 0:P], rhs=wrm[:],
                             start=True, stop=True)

        # --- chunk input staging ---
        xch = {}

        def stage_dma(c, names=("q", "k", "v"), split=1, eng=None):
            chs = xch.setdefault(c, {})
            for name in names:
                t = xin.tile([P, 8, QC], F16, tag=f"x{name}", name=f"x_{name}{c}")
                w = 8 // split
                e = eng or nc.sync
                for s in range(split):
                    e.dma_start(out=t[:, s * w:(s + 1) * w, :],
                                in_=xrs[name][:, c, s * w:(s + 1) * w, :])
                chs[name] = t

        # --- projections for chunk c (Q, K, V) ---
        def emit_proj_qk(c):
            tsl = slice(c * QC, (c + 1) * QC)
            chs = xch[c]
            for which, w_sb, dst in (("q", wq_sb, Qt), ("k", wk_sb, Kt)):
                xt = chs.pop(which)
                for et in range(2):
                    ps = gps.tile([P, QC], F32, tag="gp", name=f"pj_{which}{c}{et}")
                    for ft in range(8):
                        nc.tensor.matmul(
                            ps[:],
                            lhsT=w_sb[:, ft, et * P:(et + 1) * P],
                            rhs=xt[:, ft, :],
                            start=(ft == 0), stop=(ft == 7))
                    if which == "q":
                        nc.vector.tensor_scalar_add(
                            out=dst[:, et, tsl], in0=ps[:],
                            scalar1=bq_sb[:, et:et + 1])
                    else:
                        nc.vector.tensor_copy(out=dst[:, et, tsl], in_=ps[:])

        def emit_proj_v(c):
            xt = xch[c].pop("v")
            for tt in range(4):
                ktg = c * 4 + tt
                ps = gps.tile([P, EL], F32, tag="gp", name=f"pj_v{ktg}")
                for ft in range(8):
                    nc.tensor.matmul(
                        ps[:],
                        lhsT=xt[:, ft, tt * P:(tt + 1) * P],
                        rhs=wv_sb[:, ft, :],
                        start=(ft == 0), stop=(ft == 7))
                nc.vector.tensor_copy(
                    out=Vaug[:, ktg, :, 0:DK],
                    in_=ps.rearrange("p (h e) -> p h e", h=HL)[:, :, :])

        def emit_proj(c):
            emit_proj_qk(c)
            emit_proj_v(c)
            xch.pop(c)

        # --- output projection for chunk c ---
        def emit_yproj(c, xTt):
            for mt in range(NMT):
                yp = gps.tile([P, QC], F32, tag="gp", name=f"yp{c}{mt}")
                for ct in range(2):
                    nc.tensor.matmul(
                        yp[:],
                        lhsT=wo_sb[:, ct, mt * P:(mt + 1) * P],
                        rhs=xTt[:, ct, :],
                        start=(ct == 0), stop=(ct == 1))
                ys = ysb.tile([P, QC], F16, tag="ys", name=f"ys{c}{mt}")
                nc.vector.tensor_copy(out=ys[:], in_=yp[:])
                nc.sync.dma_start(
                    out=yO[(mt * NQC + c) * P:(mt * NQC + c + 1) * P, :],
                    in_=ys[:])

        # --- attention for chunk qc ---
        # softmax renormalize: xT_h = av[e] * recip(av[ones-row]).  Per head
        # pair, the two denominator rows are gathered (tiny DMAs) onto
        # partitions 0-1 of one tile, the reciprocal runs as one batched
        # Ln + Exp(-x) pair on ScalarE, and the recip rows bounce through
        # DRAM to partition-broadcast back to 64 lanes for the DVE multiply.
        def emit_attn(qc):
            tiles = plan[qc]
            xTt = xtp.tile([P, 2, QC], F16, tag="xT", name=f"xT{qc}")
            qsl = slice(qc * QC, (qc + 1) * QC)
            for hp in range(HL // 2):
                et = hp
                av2 = avy.tile([P, QC], F32, tag="avy", name=f"av{qc}{hp}")
                ptbs = []

                def emit_av(ti, t):
                    # the two heads' V matmuls are col-tiled onto disjoint
                    # 64-row output halves of one psum bank and run
                    # concurrently on the PE array
                    for hh in range(2):
                        nc.tensor.matmul(
                            av2[hh * DK:(hh + 1) * DK, t.s0:t.s1],
                            lhsT=Vaug[:, t.kt, 2 * hp + hh, 0:DK],
                            rhs=ptbs[ti][:, hh, t.s0:t.s1],
                            start=t.first, stop=t.last,
                            skip_group_check=True)

                for ti, t in enumerate(tiles):
                    ps = spool.tile([P, 2 * QC], F32, tag="s",
                                    name=f"s{qc}{hp}{ti}")
                    for hh in range(2):
                        po = hh * DK
                        nc.tensor.matmul(
                            ps[:, hh * QC:(hh + 1) * QC],
                            lhsT=Kt[po:po + DK, et,
                                    t.kt * KT:(t.kt + 1) * KT],
                            rhs=Qt[po:po + DK, et, qsl],
                            start=True, stop=True)
                    if ti > 0:
                        emit_av(ti - 1, tiles[ti - 1])
                    ptb = ptp.tile([P, 2, QC], F16, tag="pt",
                                   name=f"pt{qc}{hp}{ti}")
                    ptbs.append(ptb)
                    nc.scalar.activation(
                        out=ptb.rearrange("p a c -> p (a c)")[:, :],
                        in_=ps[:],
                        func=mybir.ActivationFunctionType.Exp,
                        scale=float(SCALE))
                    for hh in range(2):
                        for qt, pid in t.muls:
                            sl = slice(qt * KT, (qt + 1) * KT)
                            nc.vector.tensor_tensor(
                                out=ptb[:, hh, sl],
                                in0=ptb[:, hh, sl],
                                in1=pat_sb[:, pid, :],
                                op=mybir.AluOpType.mult)
                emit_av(len(tiles) - 1, tiles[-1])
                # denominators: ones-column matmuls over the retained exp'd
                # probabilities, accumulated into psum rows by k-tile parity
                # (four concurrent col-groups per PE slot when the first two
                # tiles span the full chunk; plain two-row accumulation
                # otherwise), then DVE adds/copies fold the parities.
                dnp = avy.tile([P, QC], F32, tag="avy", name=f"dnp{qc}{hp}")
                four = (len(tiles) >= 2
                        and all(tiles[i].s0 == 0 and tiles[i].s1 == QC
                                for i in (0, 1)))
                npar = 2 if four else 1
                par_last = {}
                for ti in range(len(tiles)):
                    par_last[ti % npar] = ti
                for ti, t in enumerate(tiles):
                    for hh in range(2):
                        row = 64 * (ti % npar) + 32 * hh
                        nc.tensor.matmul(
                            dnp[row:row + 1, t.s0:t.s1],
                            lhsT=ones_col[:, 0:1],
                            rhs=ptbs[ti][:, hh, t.s0:t.s1],
                            start=(ti < npar), stop=(ti == par_last[ti % npar]),
                            skip_group_check=True,
                            tile_position=(0, row))
                da = nrm.tile([P, QC], F32, tag="da", name=f"da{qc}{hp}")
                for hh in range(2):
                    if four:
                        do = nrm.tile([P, QC], F32, tag="do",
                                      name=f"do{qc}{hp}{hh}")
                        nc.vector.tensor_copy(
                            out=do[32 * hh:32 * hh + 1, :],
                            in_=dnp[64 + 32 * hh:64 + 32 * hh + 1, :])
                        nc.vector.tensor_tensor(
                            out=da[32 * hh:32 * hh + 1, :],
                            in0=dnp[32 * hh:32 * hh + 1, :],
                            in1=do[32 * hh:32 * hh + 1, :],
                            op=mybir.AluOpType.add)
                    else:
                        nc.vector.tensor_copy(
                            out=da[32 * hh:32 * hh + 1, :],
                            in_=dnp[32 * hh:32 * hh + 1, :])
                cps = []
                for hh in range(2):
                    cp = nrm.tile([P, QC], F32, tag="cp",
                                  name=f"cp{qc}{2 * hp + hh}")
                    nc.vector.tensor_copy(out=cp[0:DK, :],
                                          in_=av2[hh * DK:(hh + 1) * DK, :])
                    cps.append(cp)
                if qc == NQC - 1 and hp == HL // 2 - 1:
                    # final pair: every DMA hop here is exposed span, and the
                    # idle gap would drop the PE to half clock for the last
                    # output projection.  Reciprocal rows stay on partition
                    # 64 and broadcast via a K=1 matmul (ones (x) recip_row)
                    # into a transient psum tile instead of bouncing through
                    # DRAM.
                    # keep the PE HAM clock warm through the normalize
                    # chain so the last output projection runs at full rate
                    wps2 = avy.tile([P, QC], F32, tag="avy", name="warmfin")
                    for _ in range(8):
                        nc.tensor.matmul(wps2[:], lhsT=wrm[:, 0:P],
                                         rhs=wrm[:], start=True, stop=True)
                    bcp = gps.tile([P, QC], F32, tag="gp", name="bcfin")
                    for hh in range(2):
                        rl = nrm.tile([P, QC], F32, tag="dn",
                                      name=f"rl{qc}{hh}")
                        rr = nrm.tile([P, QC], F32, tag="dr",
                                      name=f"rr{qc}{hh}")
                        nc.scalar.activation(
                            out=rl[DK:DK + 1, :],
                            in_=da[32 * hh:32 * hh + 1, :],
                            func=mybir.ActivationFunctionType.Ln)
                        nc.scalar.activation(
                            out=rr[DK:DK + 1, :], in_=rl[DK:DK + 1, :],
                            func=mybir.ActivationFunctionType.Exp, scale=-1.0)
                        nc.tensor.matmul(
                            bcp[hh * DK:(hh + 1) * DK, :],
                            lhsT=ones_f32[DK:DK + 1, 0:DK],
                            rhs=rr[DK:DK + 1, :],
                            start=True, stop=True)
                        off = 0 if hh == 0 else DK
                        nc.vector.tensor_tensor(
                            out=xTt[off:off + DK, et, :],
                            in0=cps[hh][0:DK, :],
                            in1=bcp[hh * DK:(hh + 1) * DK, :],
                            op=mybir.AluOpType.mult)
                    return xTt
                dn2 = nrm.tile([2, QC], F32, tag="dn", name=f"dn{qc}{hp}")
                dr2 = nrm.tile([2, QC], F32, tag="dr", name=f"dr{qc}{hp}")
                for hh in range(2):
                    nc.sync.dma_start(out=dn2[hh:hh + 1, :],
                                      in_=da[32 * hh:32 * hh + 1, :])
                nc.scalar.activation(out=dr2[0:2, :], in_=dn2[0:2, :],
                                     func=mybir.ActivationFunctionType.Ln)
                nc.scalar.activation(out=dn2[0:2, :], in_=dr2[0:2, :],
                                     func=mybir.ActivationFunctionType.Exp,
                                     scale=-1.0)
                dnb = dbp.tile([2, QC], F32, tag="dnb", name=f"dnb{qc}{hp}")
                nc.sync.dma_start(out=dnb[:], in_=dn2[0:2, :])
                for hh in range(2):
                    bc = nrm.tile([P, QC], F32, tag="bc",
                                  name=f"bc{qc}{2 * hp + hh}")
                    nc.sync.dma_start(
                        out=bc[0:DK, :],
                        in_=dnb[hh:hh + 1, :].partition_broadcast(DK))
                    off = 0 if hh == 0 else DK
                    nc.vector.tensor_tensor(
                        out=xTt[off:off + DK, et, :], in0=cps[hh][0:DK, :],
                        in1=bc[0:DK, :], op=mybir.AluOpType.mult)
            return xTt

        # --- the pipelined schedule ---
        # chunk 0: q and k stream in (split, so projections start at the
        # half-way mark) before v competes for HBM bandwidth; scores of
        # chunk 0 feed ScalarE as early as possible.
        stage_dma(0, names=("q",), split=4, eng=nc.scalar)
        stage_dma(0, names=("k",), split=4, eng=nc.scalar)
        emit_proj_qk(0)
        stage_dma(0, names=("v",), split=2, eng=nc.scalar)
        stage_dma(1)
        emit_proj_v(0)
        xch.pop(0)
        for qc in range(NQC):
            if qc + 2 < NQC:
                stage_dma(qc + 2)
            xTt = emit_attn(qc)
            if qc + 1 < NQC:
                emit_proj(qc + 1)
            emit_yproj(qc, xTt)

    _legalize_waits(nc)
    # Raw Bass skips Bacc's codegen_inst_isa_subclasses pass; without it the
    # extended GpSimd instructions (partition_broadcast, the library reload)
    # reach walrus with empty .instr bytes -> "ISA wrong length".
    from concourse.library_overlay import lower_extended_insts
    lower_extended_insts(nc)
    return nc


# ----- SPMD runner ----------------------------------------------------------
# run_bass_kernel_spmd's axon path lowers through jax.jit(shard_map(...)),
# which this jax version emits as `call`-indirect HLO that the bass_exec
# compile hook rejects, and a single 8-replica launch isn't reachable from
# here.  Instead: one single-device jit per core (clean single-computation
# HLO), dispatched asynchronously on all 8 cores.  The NEFF is memoized by
# HLO bytes so walrus runs once, not 8 times.
_NEFF_MEMO = {}


def _install_ldw_opt():
    """walrus ships with --enable-ldw-opt=false hardcoded; without the pass
    every matmul serializes behind its LDWEIGHTS (~100ns per MM, ~65us over
    this kernel).  Rewrite the flag for our walrus invocations only."""
    from concourse import bass_utils
    inner = bass_utils.run_command
    if getattr(inner, "_is_ldw_hook", False):
        return

    def hook(cmd, *a, **kw):
        if (os.environ.get("BASS_LDW_OPT", "0") != "0"
                and any("walrus_driver" in str(c) for c in cmd[:1])):
            cmd = ["--enable-ldw-opt=true" if str(c) == "--enable-ldw-opt=false"
                   else c for c in cmd]
        return inner(cmd, *a, **kw)

    hook._is_ldw_hook = True
    bass_utils.run_command = hook


def _install_memo_hook():
    import libneuronxla
    from concourse.bass2jax import install_neuronx_cc_hook

    _install_ldw_opt()
    install_neuronx_cc_hook()
    inner = libneuronxla.neuronx_cc
    if getattr(inner, "_is_memo_hook", False):
        return

    def memo_hook(code, code_format, platform_version, file_prefix):
        import hashlib
        key = hashlib.sha256(bytes(code)).hexdigest()
        if key not in _NEFF_MEMO:
            _NEFF_MEMO[key] = inner(code, code_format, platform_version,
                                    file_prefix)
        return _NEFF_MEMO[key]

    memo_hook._is_memo_hook = True
    libneuronxla.neuronx_cc = memo_hook


def run_spmd(nc, in_maps):
    import jax
    from concourse.bass2jax import _bass_exec_p

    _install_memo_hook()
    n_cores = len(in_maps)
    partition_name = (nc.partition_id_tensor.name
                      if nc.partition_id_tensor is not None else None)
    in_names, out_names, out_avals = [], [], []
    for alloc in nc.m.functions[0].allocations:
        if not isinstance(alloc, mybir.MemoryLocationSet):
            continue
        name = alloc.memorylocations[0].name
        if alloc.kind == "ExternalInput":
            if name != partition_name:
                in_names.append(name)
        elif alloc.kind == "ExternalOutput":
            out_names.append(name)
            out_avals.append(jax.core.ShapedArray(
                tuple(alloc.tensor_shape), mybir.dt.np(alloc.dtype)))
    bind_in_names = tuple(in_names +
                          ([partition_name] if partition_name else []))

    def _body(*args):
        return tuple(_bass_exec_p.bind(
            *args, out_avals=tuple(out_avals), in_names=bind_in_names,
            out_names=tuple(out_names), lowering_input_output_aliases=(),
            sim_require_finite=True, sim_require_nnan=True, nc=nc))

    devices = jax.devices()[:n_cores]
    f = jax.jit(_body)
    futs = []
    for c in range(n_cores):
        args = [jax.device_put(np.asarray(in_maps[c][nm]), devices[c])
                for nm in in_names]
        if partition_name:
            args.append(jax.device_put(np.array([[c]], np.uint32), devices[c]))
        futs.append(f(*args))
    return [{nm: np.asarray(futs[c][i]) for i, nm in enumerate(out_names)}
            for c in range(n_cores)]


# ----- host wrapper ---------------------------------------------------------
_CACHE = {}


def _get_program(mask):
    key = mask.tobytes()
    if key not in _CACHE:
        plan, pats = _mask_plan(mask)
        nc = build_program(plan, pats.shape[0])
        _CACHE[key] = (nc, pats)
    return _CACHE[key]


def _chunk_major(xT):
    """[D, S] f32 -> [P, NQC*8*QC] f16 chunk-major for contiguous chunk DMA"""
    return np.ascontiguousarray(
        xT.astype(F16NP).reshape(8, P, NQC, QC).transpose(1, 2, 0, 3)
        .reshape(P, NQC * 8 * QC))


def make_in_maps(q, k, v, mask, wq, bq, wk, bk, wv, bv, wo, bo, pats):
    q, k, v = (np.asarray(a, np.float32) for a in (q, k, v))
    in_maps = []
    for c in range(NCORES):
        b, g = divmod(c, TP)
        sl = slice(g * EL, (g + 1) * EL)
        in_maps.append({
            "xq2": _chunk_major(q[b].T),
            "xk2": _chunk_major(k[b].T),
            "xv2": _chunk_major(v[b].T),
            "wqT": np.ascontiguousarray(wq[sl, :].T.astype(F16NP)),
            "wkT": np.ascontiguousarray(wk[sl, :].T.astype(F16NP)),
            "wvT": np.ascontiguousarray(wv[sl, :].T.astype(F16NP)),
            "woT": np.ascontiguousarray(wo[:, sl].T).astype(F16NP),
            "bq2": np.ascontiguousarray(bq[sl].reshape(2, P)),
            "pats": pats,
        })
    return in_maps


def assemble_output(results, bv, wo, bo):
    extra = (np.asarray(bv, np.float64) @ np.asarray(wo, np.float64).T
             + np.asarray(bo, np.float64)).astype(np.float32)  # [D]
    y = np.empty((B, S, D), np.float32)
    for b in range(B):
        acc = np.zeros((D, S), np.float32)
        for g in range(TP):
            yT = (results[b * TP + g]["yO"].astype(np.float32)
                  .reshape(NMT, NQC, P, QC).transpose(0, 2, 1, 3)
                  .reshape(D, S))
            acc += yT
        y[b] = acc.T + extra[None, :]
    return y


def kernel(q, k, v, mask, wq, bq, wk, bk, wv, bv, wo, bo):
    mask2d = np.asarray(mask).reshape(S, S)
    nc, pats = _get_program(mask2d)
    in_maps = make_in_maps(q, k, v, mask2d, wq, bq, wk, bk, wv, bv, wo, bo, pats)
    return assemble_output(run_spmd(nc, in_maps), bv, wo, bo)


# revision 26
# speedup vs baseline: 1.0826x; 1.0826x over previous
"""Multi-head attention (B=2, S=2048, D=1024, H=16, causal mask) on 8 trn2
NeuronCores.

Sharding: 2-way data parallel over batch x 4-way tensor parallel over head
groups (4 heads / core).  Core c handles batch c//4, head group c%4.

Everything on-chip lives feature-major ("transposed") so no transposes are
ever needed: inputs arrive chunk-major [P, chunk, ftile, t] in fp16; Q/K
projections produce Qh^T/Kh^T [e, t]; scores come out keys-major [k, q];
exp(p) feeds A@V directly as the moving operand with V (+ a ones column
that makes the softmax denominator fall out of the same matmul)
stationary; the attention output appears as x_att^T [e, q], which is
exactly the layout the output projection wants.  Each core emits its y^T
partial and the host sums the 4 partials of each batch group (row-parallel
tensor-parallel reduction).

Differences vs the phase-separated v1:
  * single software-pipelined loop over query chunks: the projection
    matmuls of chunk c+1 fill the tensor-engine gaps left while ScalarE
    runs the exp stream of chunk c (the Tile scheduler interleaves by
    priority/readiness; ScalarE exp is the critical resource at ~92us).
  * K bias dropped entirely (adding bk shifts every score column by a
    per-query constant, which softmax cancels exactly).
  * V bias and output bias are applied analytically on the host
    (bv @ wo.T + bo added once after the partial-sum gather).
  * softmax denominator reciprocal as a batched 2-row Ln + Exp(-x) pair
    on ScalarE per head pair; the recip rows partition-broadcast through
    a DRAM bounce mid-kernel, and via a K=1 ones matmul on the (otherwise
    idle) PE for the final pair so the last output projection is not
    delayed and stays on a warm clock.
  * exp always runs full-width [128, 2*QC] (one ACTIVATE per k-tile per
    head pair); scores matmuls write the full query range so the psum is
    fully initialized.  Mask patterns zero the dead blocks after exp.
  * junk matmuls warm the PE HAM clock gate while the first input chunk
    streams in (and again under the final normalize chain).
  * y partials stream out as fp16 directly from a psum->sbuf cast.

Mask handled generically: the [S,S] mask is classified on the host into
128x128 blocks (zero / one / mixed).  Zero k-tiles are skipped entirely,
mixed blocks get a pattern-multiply after exp with deduplicated patterns
uploaded as data.
"""

import os
import sys

import numpy as np

for _p in ("/opt/trn_rl_repo", "/root/.axon_site/_ro/trn_rl_repo"):
    if os.path.isdir(_p) and _p not in sys.path:
        sys.path.append(_p)

import ml_dtypes  # noqa: E402
from contextlib import ExitStack  # noqa: E402

import concourse.bass as bass  # noqa: E402
import concourse.tile as tile  # noqa: E402
from concourse import mybir  # noqa: E402

# ----- problem constants (hardcoded per contract) ---------------------------
B, S, D, H, DK = 2, 2048, 1024, 16, 64
NCORES = 8
TP = 4                      # head-parallel ways (per batch group)
EL = D // TP                # 256 local head dims = 4 heads
HL = H // TP                # 4 local heads
QC = 512                    # query-chunk (columns per attention pass)
NQC = S // QC               # 4
KT = 128                    # key tile (contraction tile for A@V)
NKT = S // KT               # 16
P = 128
NMT = D // P                # 8 output-feature tiles
SCALE = 1.0 / np.sqrt(DK)

F32 = mybir.dt.float32
F16 = mybir.dt.float16
F16NP = np.float16


# ----- host-side mask analysis ---------------------------------------------
class _KTile:
    __slots__ = ("kt", "s0", "s1", "muls", "first", "last")

    def __init__(self, kt, s0, s1, muls):
        self.kt, self.s0, self.s1, self.muls = kt, s0, s1, muls
        self.first = False
        self.last = False


def _mask_plan(mask2d):
    """mask2d: [S, S] ints, mask2d[q, k] (1 = attend).  Returns
    (plan, patterns) where plan[qc] is a list of _KTile and patterns is a
    f16 array [n_pat, 128, 128] of transposed (k-major) mask blocks."""
    mT = (mask2d != 0).astype(np.float32).T          # [k, q]
    nqt = S // KT
    blk = mT.reshape(NKT, KT, nqt, KT).transpose(0, 2, 1, 3)  # [kt, qt, 128, 128]
    sums = blk.sum(axis=(2, 3))
    patterns = []
    pat_idx = {}

    def pattern_id(kt, qt):
        key = blk[kt, qt].tobytes()
        if key not in pat_idx:
            pat_idx[key] = len(patterns)
            patterns.append(blk[kt, qt].astype(np.float16))
        return pat_idx[key]

    qt_per_qc = QC // KT
    plan = []
    for qc in range(NQC):
        tiles = []
        for kt in range(NKT):
            sub = sums[kt, qc * qt_per_qc:(qc + 1) * qt_per_qc]
            nz = [i for i in range(qt_per_qc) if sub[i] > 0]
            if not nz:
                continue
            s0, s1 = nz[0] * KT, (nz[-1] + 1) * KT
            tiles.append(_KTile(kt, s0, s1, None))
        if not tiles:
            raise ValueError(f"query chunk {qc} has no unmasked keys")
        u0 = min(t.s0 for t in tiles)
        u1 = max(t.s1 for t in tiles)
        tiles[0].s0, tiles[0].s1 = u0, u1
        tiles[0].first = True
        tiles[-1].last = True
        for t in tiles:
            muls = []
            for qt in range(t.s0 // KT, t.s1 // KT):
                full = sums[t.kt, qc * qt_per_qc + qt]
                if full != KT * KT:          # zero or mixed -> needs pattern
                    muls.append((qt, pattern_id(t.kt, qc * qt_per_qc + qt)))
            t.muls = muls
        plan.append(tiles)
    pats = np.stack(patterns) if patterns else np.zeros((1, KT, KT), np.float16)
    return plan, pats


# ----- TileContext with a codegen-safe exit drain ---------------------------
# The stock kernel-tail drain carries one semaphore wait per engine/queue the
# kernel touched; CoreV3 codegen rejects instructions with more than two
# waits ("Too many sync wait commands").  Split the waits across preceding
# sync-engine nops, two per instruction, so the drain itself needs none.
class _TileContext(tile.TileContext):
    def _drain_and_barrier(self, tick_clock, wait_clock):
        from concourse.vector_clock import ScopedClock
        nc = self.nc
        probe = nc.sync.nop()
        wait_clock.add_sem_waits(
            probe.ins, ScopedClock({None: tick_clock.global_clock}))
        si = probe.ins.sync_info
        waits = list(si.on_wait) if si and si.on_wait else []
        if len(waits) > 1:
            probe.ins.sync_info = mybir.SyncInfo(
                on_wait=waits[:1], on_update=list(si.on_update or []))
            for w in waits[1:]:
                n = nc.sync.nop()
                n.ins.sync_info = mybir.SyncInfo(on_wait=[w], on_update=[])
        nc.sync.drain()
        nc.all_engine_barrier()
        assert self.sems is not None
        popped = nc._tile_sem_poison_stack.pop()
        assert popped is self._sem_poison
        nc.clear_and_free_semaphores(list(self.sems.allocated().values()))
        nc.all_engine_barrier()


# The same wait-count limit applies to ordinary engine instructions under
# this walrus build, so after the program is fully built, hoist all but one
# wait of every instruction onto preceding same-engine no-ops.
def _legalize_waits(nc, limit=1):
    for bb in nc.main_func.blocks:
        insts = list(bb.instructions)
        out = []
        for inst in insts:
            si = inst.sync_info
            waits = list(si.on_wait) if si and si.on_wait else []
            if len(waits) > limit:
                for w in waits[:-limit]:
                    nop = mybir.InstNoOp(
                        name=nc.get_next_instruction_name(), ins=[], outs=[])
                    nop.engine = inst.engine
                    nop.sync_info = mybir.SyncInfo(on_wait=[w], on_update=[])
                    nc.register_instruction(nop, overwrite=True)
                    out.append(nop)
                inst.sync_info = mybir.SyncInfo(
                    on_wait=waits[-limit:],
                    on_update=list(si.on_update or []))
            out.append(inst)
        bb.instructions = out


# ----- the bass program -----------------------------------------------------
def build_program(plan, n_pat):
    nc = bass.Bass(num_devices=NCORES)

    xq2 = nc.dram_tensor("xq2", [P, NQC * 8 * QC], F16, kind="ExternalInput")
    xk2 = nc.dram_tensor("xk2", [P, NQC * 8 * QC], F16, kind="ExternalInput")
    xv2 = nc.dram_tensor("xv2", [P, NQC * 8 * QC], F16, kind="ExternalInput")
    wqT = nc.dram_tensor("wqT", [D, EL], F16, kind="ExternalInput")
    wkT = nc.dram_tensor("wkT", [D, EL], F16, kind="ExternalInput")
    wvT = nc.dram_tensor("wvT", [D, EL], F16, kind="ExternalInput")
    woT = nc.dram_tensor("woT", [EL, D], F16, kind="ExternalInput")
    bq2 = nc.dram_tensor("bq2", [2, P], F32, kind="ExternalInput")
    pats = nc.dram_tensor("pats", [n_pat, KT, KT], F16, kind="ExternalInput")
    yO = nc.dram_tensor("yO", [NMT * NQC * P, QC], F16, kind="ExternalOutput")

    xrs = {
        "q": xq2.rearrange("p (c a t) -> p c a t", c=NQC, a=8),
        "k": xk2.rearrange("p (c a t) -> p c a t", c=NQC, a=8),
        "v": xv2.rearrange("p (c a t) -> p c a t", c=NQC, a=8),
    }

    with ExitStack() as ctx:
        tc = ctx.enter_context(_TileContext(nc))
        singles = ctx.enter_context(tc.tile_pool(name="singles", bufs=1))

        # --- persistent SBUF state ---
        wq_sb = singles.tile([P, 8, EL], F16)
        wk_sb = singles.tile([P, 8, EL], F16)
        wv_sb = singles.tile([P, 8, EL], F16)
        wo_sb = singles.tile([P, 2, D], F16)
        nc.sync.dma_start(out=wq_sb[:], in_=wqT.rearrange("(a p) e -> p a e", p=P))
        nc.sync.dma_start(out=wk_sb[:], in_=wkT.rearrange("(a p) e -> p a e", p=P))
        nc.sync.dma_start(out=wv_sb[:], in_=wvT.rearrange("(a p) e -> p a e", p=P))
        nc.sync.dma_start(out=wo_sb[:], in_=woT.rearrange("(a p) m -> p a m", p=P))
        bq_sb = singles.tile([P, 2], F32)
        nc.sync.dma_start(out=bq_sb[:], in_=bq2.rearrange("a p -> p a"))
        pat_sb = singles.tile([P, n_pat, KT], F16)
        nc.sync.dma_start(out=pat_sb[:], in_=pats.rearrange("n p k -> p n k"))

        Qt = singles.tile([P, 2, S], F16)     # [e-within-tile, e-tile, t]
        Kt = singles.tile([P, 2, S], F16)
        Vaug = singles.tile([P, NKT, HL, DK + 1], F16)  # [t-in-ktile, kt, h, e|1]
        nc.vector.memset(Vaug[:, :, :, DK:DK + 1], 1.0)
        ones_f32 = singles.tile([P, DK], F32)
        nc.vector.memset(ones_f32[:], 1.0)
        ones_col = singles.tile([P, 1], F16)
        nc.vector.memset(ones_col[:], 1.0)

        # warm the Exp activation table while the inputs stream in
        warms = singles.tile([1, 8], F32)
        nc.scalar.activation(out=warms[0:1, 0:1], in_=bq_sb[0:1, 0:1],
                             func=mybir.ActivationFunctionType.Exp)

        # --- pools ---
        xin = ctx.enter_context(tc.tile_pool(name="xin", bufs=2))
        spool = ctx.enter_context(tc.tile_pool(name="spool", bufs=2, space="PSUM"))
        avy = ctx.enter_context(tc.tile_pool(name="avy", bufs=2, space="PSUM"))
        gps = ctx.enter_context(tc.tile_pool(name="gps", bufs=2, space="PSUM"))
        ptp = ctx.enter_context(tc.tile_pool(name="ptp", bufs=4))
        xtp = ctx.enter_context(tc.tile_pool(name="xtp", bufs=2))
        nrm = ctx.enter_context(tc.tile_pool(name="nrm", bufs=4))
        ysb = ctx.enter_context(tc.tile_pool(name="ysb", bufs=3))
        dbp = ctx.enter_context(tc.tile_pool(name="dbp", bufs=2, space="DRAM"))

        # --- PE HAM warm-up: junk matmuls while the first chunk loads ---
        wrm = singles.tile([P, QC], F16)
        nc.vector.memset(wrm[:], 0.0)
        wps = gps.tile([P, QC], F32, tag="gp", name="warmps")
        for _ in range(10):
            nc.tensor.matmul(wps[:], lhsT=wrm[:, 0:P], rhs=wrm[:],
                             start=True, stop=True)

        # --- chunk input staging ---
        xch = {}

        def stage_dma(c, names=("q", "k", "v"), split=1, eng=None):
            chs = xch.setdefault(c, {})
            for name in names:
                t = xin.tile([P, 8, QC], F16, tag=f"x{name}", name=f"x_{name}{c}")
                w = 8 // split
                e = eng or nc.sync
                for s in range(split):
                    e.dma_start(out=t[:, s * w:(s + 1) * w, :],
                                in_=xrs[name][:, c, s * w:(s + 1) * w, :])
                chs[name] = t

        # --- projections for chunk c (Q, K, V) ---
        def emit_proj_qk(c):
            tsl = slice(c * QC, (c + 1) * QC)
            chs = xch[c]
            for which, w_sb, dst in (("q", wq_sb, Qt), ("k", wk_sb, Kt)):
                xt = chs.pop(which)
                for et in range(2):
                    ps = gps.tile([P, QC], F32, tag="gp", name=f"pj_{which}{c}{et}")
                    for ft in range(8):
                        nc.tensor.matmul(
                            ps[:],
                            lhsT=w_sb[:, ft, et * P:(et + 1) * P],
                            rhs=xt[:, ft, :],
                            start=(ft == 0), stop=(ft == 7))
                    if which == "q":
                        nc.vector.tensor_scalar_add(
                            out=dst[:, et, tsl], in0=ps[:],
                            scalar1=bq_sb[:, et:et + 1])
                    else:
                        nc.vector.tensor_copy(out=dst[:, et, tsl], in_=ps[:])

        def emit_proj_v(c):
            xt = xch[c].pop("v")
            for tt in range(4):
                ktg = c * 4 + tt
                ps = gps.tile([P, EL], F32, tag="gp", name=f"pj_v{ktg}")
                for ft in range(8):
                    nc.tensor.matmul(
                        ps[:],
                        lhsT=xt[:, ft, tt * P:(tt + 1) * P],
                        rhs=wv_sb[:, ft, :],
                        start=(ft == 0), stop=(ft == 7))
                nc.vector.tensor_copy(
                    out=Vaug[:, ktg, :, 0:DK],
                    in_=ps.rearrange("p (h e) -> p h e", h=HL)[:, :, :])

        def emit_proj(c):
            emit_proj_qk(c)
            emit_proj_v(c)
            xch.pop(c)

        # --- output projection for chunk c ---
        def emit_yproj(c, xTt):
            for mt in range(NMT):
                yp = gps.tile([P, QC], F32, tag="gp", name=f"yp{c}{mt}")
                for ct in range(2):
                    nc.tensor.matmul(
                        yp[:],
                        lhsT=wo_sb[:, ct, mt * P:(mt + 1) * P],
                        rhs=xTt[:, ct, :],
                        start=(ct == 0), stop=(ct == 1))
                ys = ysb.tile([P, QC], F16, tag="ys", name=f"ys{c}{mt}")
                nc.vector.tensor_copy(out=ys[:], in_=yp[:])
                nc.sync.dma_start(
                    out=yO[(mt * NQC + c) * P:(mt * NQC + c + 1) * P, :],
                    in_=ys[:])

        # --- attention for chunk qc ---
        # softmax renormalize: xT_h = av[e] * recip(av[ones-row]).  Per head
        # pair, the two denominator rows are gathered (tiny DMAs) onto
        # partitions 0-1 of one tile, the reciprocal runs as one batched
        # Ln + Exp(-x) pair on ScalarE, and the recip rows bounce through
        # DRAM to partition-broadcast back to 64 lanes for the DVE multiply.
        def emit_attn(qc):
            tiles = plan[qc]
            xTt = xtp.tile([P, 2, QC], F16, tag="xT", name=f"xT{qc}")
            qsl = slice(qc * QC, (qc + 1) * QC)
            for hp in range(HL // 2):
                et = hp
                avs = [avy.tile([P, QC], F32, tag="avy",
                                name=f"av{qc}{2 * hp + hh}")
                       for hh in range(2)]
                ptbs = []

                def emit_av(ti, t):
                    for hh in range(2):
                        nc.tensor.matmul(
                            avs[hh][0:DK + 1, t.s0:t.s1],
                            lhsT=Vaug[:, t.kt, 2 * hp + hh, :],
                            rhs=ptbs[ti][:, hh, t.s0:t.s1],
                            start=t.first, stop=t.last,
                            skip_group_check=True)

                for ti, t in enumerate(tiles):
                    ps = spool.tile([P, 2 * QC], F32, tag="s",
                                    name=f"s{qc}{hp}{ti}")
                    for hh in range(2):
                        po = hh * DK
                        nc.tensor.matmul(
                            ps[:, hh * QC:(hh + 1) * QC],
                            lhsT=Kt[po:po + DK, et,
                                    t.kt * KT:(t.kt + 1) * KT],
                            rhs=Qt[po:po + DK, et, qsl],
                            start=True, stop=True)
                    if ti > 0:
                        emit_av(ti - 1, tiles[ti - 1])
                    ptb = ptp.tile([P, 2, QC], F16, tag="pt",
                                   name=f"pt{qc}{hp}{ti}")
                    ptbs.append(ptb)
                    nc.scalar.activation(
                        out=ptb.rearrange("p a c -> p (a c)")[:, :],
                        in_=ps[:],
                        func=mybir.ActivationFunctionType.Exp,
                        scale=float(SCALE))
                    for hh in range(2):
                        for qt, pid in t.muls:
                            sl = slice(qt * KT, (qt + 1) * KT)
                            nc.vector.tensor_tensor(
                                out=ptb[:, hh, sl],
                                in0=ptb[:, hh, sl],
                                in1=pat_sb[:, pid, :],
                                op=mybir.AluOpType.mult)
                emit_av(len(tiles) - 1, tiles[-1])
                cps = []
                for hh in range(2):
                    cp = nrm.tile([P, QC], F32, tag="cp",
                                  name=f"cp{qc}{2 * hp + hh}")
                    nc.vector.tensor_copy(out=cp[0:DK + 1, :],
                                          in_=avs[hh][0:DK + 1, :])
                    cps.append(cp)
                if qc == NQC - 1 and hp == HL // 2 - 1:
                    # final pair: every DMA hop here is exposed span, and the
                    # idle gap would drop the PE to half clock for the last
                    # output projection.  Reciprocal rows stay on partition
                    # 64 and broadcast via a K=1 matmul (ones (x) recip_row)
                    # into a transient psum tile instead of bouncing through
                    # DRAM.
                    # keep the PE HAM clock warm through the normalize
                    # chain so the last output projection runs at full rate
                    wps2 = avy.tile([P, QC], F32, tag="avy", name="warmfin")
                    for _ in range(8):
                        nc.tensor.matmul(wps2[:], lhsT=wrm[:, 0:P],
                                         rhs=wrm[:], start=True, stop=True)
                    bcp = gps.tile([P, QC], F32, tag="gp", name="bcfin")
                    for hh in range(2):
                        rl = nrm.tile([P, QC], F32, tag="dn",
                                      name=f"rl{qc}{hh}")
                        rr = nrm.tile([P, QC], F32, tag="dr",
                                      name=f"rr{qc}{hh}")
                        nc.scalar.activation(
                            out=rl[DK:DK + 1, :], in_=cps[hh][DK:DK + 1, :],
                            func=mybir.ActivationFunctionType.Ln)
                        nc.scalar.activation(
                            out=rr[DK:DK + 1, :], in_=rl[DK:DK + 1, :],
                            func=mybir.ActivationFunctionType.Exp, scale=-1.0)
                        nc.tensor.matmul(
                            bcp[hh * DK:(hh + 1) * DK, :],
                            lhsT=ones_f32[DK:DK + 1, 0:DK],
                            rhs=rr[DK:DK + 1, :],
                            start=True, stop=True)
                        off = 0 if hh == 0 else DK
                        nc.vector.tensor_tensor(
                            out=xTt[off:off + DK, et, :],
                            in0=cps[hh][0:DK, :],
                            in1=bcp[hh * DK:(hh + 1) * DK, :],
                            op=mybir.AluOpType.mult)
                    return xTt
                dn2 = nrm.tile([2, QC], F32, tag="dn", name=f"dn{qc}{hp}")
                dr2 = nrm.tile([2, QC], F32, tag="dr", name=f"dr{qc}{hp}")
                for hh in range(2):
                    nc.sync.dma_start(out=dn2[hh:hh + 1, :],
                                      in_=cps[hh][DK:DK + 1, :])
                nc.scalar.activation(out=dr2[0:2, :], in_=dn2[0:2, :],
                                     func=mybir.ActivationFunctionType.Ln)
                nc.scalar.activation(out=dn2[0:2, :], in_=dr2[0:2, :],
                                     func=mybir.ActivationFunctionType.Exp,
                                     scale=-1.0)
                dnb = dbp.tile([2, QC], F32, tag="dnb", name=f"dnb{qc}{hp}")
                nc.sync.dma_start(out=dnb[:], in_=dn2[0:2, :])
                for hh in range(2):
                    bc = nrm.tile([P, QC], F32, tag="bc",
                                  name=f"bc{qc}{2 * hp + hh}")
                    nc.sync.dma_start(
                        out=bc[0:DK, :],
                        in_=dnb[hh:hh + 1, :].partition_broadcast(DK))
                    off = 0 if hh == 0 else DK
                    nc.vector.tensor_tensor(
                        out=xTt[off:off + DK, et, :], in0=cps[hh][0:DK, :],
                        in1=bc[0:DK, :], op=mybir.AluOpType.mult)
            return xTt

        # --- the pipelined schedule ---
        # chunk 0: q and k stream in (split, so projections start at the
        # half-way mark) before v competes for HBM bandwidth; scores of
        # chunk 0 feed ScalarE as early as possible.
        stage_dma(0, names=("q",), split=4, eng=nc.scalar)
        stage_dma(0, names=("k",), split=4, eng=nc.scalar)
        emit_proj_qk(0)
        stage_dma(0, names=("v",), split=2, eng=nc.scalar)
        stage_dma(1)
        emit_proj_v(0)
        xch.pop(0)
        for qc in range(NQC):
            if qc + 2 < NQC:
                stage_dma(qc + 2)
            xTt = emit_attn(qc)
            if qc + 1 < NQC:
                emit_proj(qc + 1)
            emit_yproj(qc, xTt)

    _legalize_waits(nc)
    # Raw Bass skips Bacc's codegen_inst_isa_subclasses pass; without it the
    # extended GpSimd instructions (partition_broadcast, the library reload)
    # reach walrus with empty .instr bytes -> "ISA wrong length".
    from concourse.library_overlay import lower_extended_insts
    lower_extended_insts(nc)
    return nc


# ----- SPMD runner ----------------------------------------------------------
# run_bass_kernel_spmd's axon path lowers through jax.jit(shard_map(...)),
# which this jax version emits as `call`-indirect HLO that the bass_exec
# compile hook rejects, and a single 8-replica launch isn't reachable from
# here.  Instead: one single-device jit per core (clean single-computation
# HLO), dispatched asynchronously on all 8 cores.  The NEFF is memoized by
# HLO bytes so walrus runs once, not 8 times.
_NEFF_MEMO = {}


def _install_ldw_opt():
    """walrus ships with --enable-ldw-opt=false hardcoded; without the pass
    every matmul serializes behind its LDWEIGHTS (~100ns per MM, ~65us over
    this kernel).  Rewrite the flag for our walrus invocations only."""
    from concourse import bass_utils
    inner = bass_utils.run_command
    if getattr(inner, "_is_ldw_hook", False):
        return

    def hook(cmd, *a, **kw):
        if (os.environ.get("BASS_LDW_OPT", "0") != "0"
                and any("walrus_driver" in str(c) for c in cmd[:1])):
            cmd = ["--enable-ldw-opt=true" if str(c) == "--enable-ldw-opt=false"
                   else c for c in cmd]
        return inner(cmd, *a, **kw)

    hook._is_ldw_hook = True
    bass_utils.run_command = hook


def _install_memo_hook():
    import libneuronxla
    from concourse.bass2jax import install_neuronx_cc_hook

    _install_ldw_opt()
    install_neuronx_cc_hook()
    inner = libneuronxla.neuronx_cc
    if getattr(inner, "_is_memo_hook", False):
        return

    def memo_hook(code, code_format, platform_version, file_prefix):
        import hashlib
        key = hashlib.sha256(bytes(code)).hexdigest()
        if key not in _NEFF_MEMO:
            _NEFF_MEMO[key] = inner(code, code_format, platform_version,
                                    file_prefix)
        return _NEFF_MEMO[key]

    memo_hook._is_memo_hook = True
    libneuronxla.neuronx_cc = memo_hook


def run_spmd(nc, in_maps):
    import jax
    from concourse.bass2jax import _bass_exec_p

    _install_memo_hook()
    n_cores = len(in_maps)
    partition_name = (nc.partition_id_tensor.name
                      if nc.partition_id_tensor is not None else None)
    in_names, out_names, out_avals = [], [], []
    for alloc in nc.m.functions[0].allocations:
        if not isinstance(alloc, mybir.MemoryLocationSet):
            continue
        name = alloc.memorylocations[0].name
        if alloc.kind == "ExternalInput":
            if name != partition_name:
                in_names.append(name)
        elif alloc.kind == "ExternalOutput":
            out_names.append(name)
            out_avals.append(jax.core.ShapedArray(
                tuple(alloc.tensor_shape), mybir.dt.np(alloc.dtype)))
    bind_in_names = tuple(in_names +
                          ([partition_name] if partition_name else []))

    def _body(*args):
        return tuple(_bass_exec_p.bind(
            *args, out_avals=tuple(out_avals), in_names=bind_in_names,
            out_names=tuple(out_names), lowering_input_output_aliases=(),
            sim_require_finite=True, sim_require_nnan=True, nc=nc))

    devices = jax.devices()[:n_cores]
    f = jax.jit(_body)
    futs = []
    for c in range(n_cores):
        args = [jax.device_put(np.asarray(in_maps[c][nm]), devices[c])
                for nm in in_names]
        if partition_name:
            args.append(jax.device_put(np.array([[c]], np.uint32), devices[c]))
        futs.append(f(*args))
    return [{nm: np.asarray(futs[c][i]) for i, nm in enumerate(out_names)}
            for c in range(n_cores)]


# ----- host wrapper ---------------------------------------------------------
_CACHE = {}


def _get_program(mask):
    key = mask.tobytes()
    if key not in _CACHE:
        plan, pats = _mask_plan(mask)
        nc = build_program(plan, pats.shape[0])
        _CACHE[key] = (nc, pats)
    return _CACHE[key]


def _chunk_major(xT):
    """[D, S] f32 -> [P, NQC*8*QC] f16 chunk-major for contiguous chunk DMA"""
    return np.ascontiguousarray(
        xT.astype(F16NP).reshape(8, P, NQC, QC).transpose(1, 2, 0, 3)
        .reshape(P, NQC * 8 * QC))


def make_in_maps(q, k, v, mask, wq, bq, wk, bk, wv, bv, wo, bo, pats):
    q, k, v = (np.asarray(a, np.float32) for a in (q, k, v))
    in_maps = []
    for c in range(NCORES):
        b, g = divmod(c, TP)
        sl = slice(g * EL, (g + 1) * EL)
        in_maps.append({
            "xq2": _chunk_major(q[b].T),
            "xk2": _chunk_major(k[b].T),
            "xv2": _chunk_major(v[b].T),
            "wqT": np.ascontiguousarray(wq[sl, :].T.astype(F16NP)),
            "wkT": np.ascontiguousarray(wk[sl, :].T.astype(F16NP)),
            "wvT": np.ascontiguousarray(wv[sl, :].T.astype(F16NP)),
            "woT": np.ascontiguousarray(wo[:, sl].T).astype(F16NP),
            "bq2": np.ascontiguousarray(bq[sl].reshape(2, P)),
            "pats": pats,
        })
    return in_maps


def assemble_output(results, bv, wo, bo):
    extra = (np.asarray(bv, np.float64) @ np.asarray(wo, np.float64).T
             + np.asarray(bo, np.float64)).astype(np.float32)  # [D]
    y = np.empty((B, S, D), np.float32)
    for b in range(B):
        acc = np.zeros((D, S), np.float32)
        for g in range(TP):
            yT = (results[b * TP + g]["yO"].astype(np.float32)
                  .reshape(NMT, NQC, P, QC).transpose(0, 2, 1, 3)
                  .reshape(D, S))
            acc += yT
        y[b] = acc.T + extra[None, :]
    return y


def kernel(q, k, v, mask, wq, bq, wk, bk, wv, bv, wo, bo):
    mask2d = np.asarray(mask).reshape(S, S)
    nc, pats = _get_program(mask2d)
    in_maps = make_in_maps(q, k, v, mask2d, wq, bq, wk, bk, wv, bv, wo, bo, pats)
    return assemble_output(run_spmd(nc, in_maps), bv, wo, bo)


# revision 27
# speedup vs baseline: 1.1025x; 1.0184x over previous
"""Multi-head attention (B=2, S=2048, D=1024, H=16, causal mask) on 8 trn2
NeuronCores.

Sharding: 2-way data parallel over batch x 4-way tensor parallel over head
groups (4 heads / core).  Core c handles batch c//4, head group c%4.

Everything on-chip lives feature-major ("transposed") so no transposes are
ever needed: inputs arrive chunk-major [P, chunk, ftile, t] in fp16; Q/K
projections produce Qh^T/Kh^T [e, t]; scores come out keys-major [k, q];
exp(p) feeds A@V directly as the moving operand with V (+ a ones column
that makes the softmax denominator fall out of the same matmul)
stationary; the attention output appears as x_att^T [e, q], which is
exactly the layout the output projection wants.  Each core emits its y^T
partial and the host sums the 4 partials of each batch group (row-parallel
tensor-parallel reduction).

Differences vs the phase-separated v1:
  * single software-pipelined loop over query chunks: the projection
    matmuls of chunk c+1 fill the tensor-engine gaps left while ScalarE
    runs the exp stream of chunk c (the Tile scheduler interleaves by
    priority/readiness; ScalarE exp is the critical resource at ~92us).
  * K bias dropped entirely (adding bk shifts every score column by a
    per-query constant, which softmax cancels exactly).
  * V bias and output bias are applied analytically on the host
    (bv @ wo.T + bo added once after the partial-sum gather).
  * softmax denominator reciprocal as a batched 2-row Ln + Exp(-x) pair
    on ScalarE per head pair; the recip rows partition-broadcast through
    a DRAM bounce mid-kernel, and via a K=1 ones matmul on the (otherwise
    idle) PE for the final pair so the last output projection is not
    delayed and stays on a warm clock.
  * exp always runs full-width [128, 2*QC] (one ACTIVATE per k-tile per
    head pair); scores matmuls write the full query range so the psum is
    fully initialized.  Mask patterns zero the dead blocks after exp.
  * junk matmuls warm the PE HAM clock gate while the first input chunk
    streams in (and again under the final normalize chain).
  * y partials stream out as fp16 directly from a psum->sbuf cast.

Mask handled generically: the [S,S] mask is classified on the host into
128x128 blocks (zero / one / mixed).  Zero k-tiles are skipped entirely,
mixed blocks get a pattern-multiply after exp with deduplicated patterns
uploaded as data.
"""

import os
import sys

import numpy as np

for _p in ("/opt/trn_rl_repo", "/root/.axon_site/_ro/trn_rl_repo"):
    if os.path.isdir(_p) and _p not in sys.path:
        sys.path.append(_p)

import ml_dtypes  # noqa: E402
from contextlib import ExitStack  # noqa: E402

import concourse.bass as bass  # noqa: E402
import concourse.tile as tile  # noqa: E402
from concourse import mybir  # noqa: E402

# ----- problem constants (hardcoded per contract) ---------------------------
B, S, D, H, DK = 2, 2048, 1024, 16, 64
NCORES = 8
TP = 4                      # head-parallel ways (per batch group)
EL = D // TP                # 256 local head dims = 4 heads
HL = H // TP                # 4 local heads
QC = 512                    # query-chunk (columns per attention pass)
NQC = S // QC               # 4
KT = 128                    # key tile (contraction tile for A@V)
NKT = S // KT               # 16
P = 128
NMT = D // P                # 8 output-feature tiles
SCALE = 1.0 / np.sqrt(DK)

F32 = mybir.dt.float32
F16 = mybir.dt.float16
F16NP = np.float16


# ----- host-side mask analysis ---------------------------------------------
class _KTile:
    __slots__ = ("kt", "s0", "s1", "muls", "first", "last")

    def __init__(self, kt, s0, s1, muls):
        self.kt, self.s0, self.s1, self.muls = kt, s0, s1, muls
        self.first = False
        self.last = False


def _mask_plan(mask2d):
    """mask2d: [S, S] ints, mask2d[q, k] (1 = attend).  Returns
    (plan, patterns) where plan[qc] is a list of _KTile and patterns is a
    f16 array [n_pat, 128, 128] of transposed (k-major) mask blocks."""
    mT = (mask2d != 0).astype(np.float32).T          # [k, q]
    nqt = S // KT
    blk = mT.reshape(NKT, KT, nqt, KT).transpose(0, 2, 1, 3)  # [kt, qt, 128, 128]
    sums = blk.sum(axis=(2, 3))
    patterns = []
    pat_idx = {}

    def pattern_id(kt, qt):
        key = blk[kt, qt].tobytes()
        if key not in pat_idx:
            pat_idx[key] = len(patterns)
            patterns.append(blk[kt, qt].astype(np.float16))
        return pat_idx[key]

    qt_per_qc = QC // KT
    plan = []
    for qc in range(NQC):
        tiles = []
        for kt in range(NKT):
            sub = sums[kt, qc * qt_per_qc:(qc + 1) * qt_per_qc]
            nz = [i for i in range(qt_per_qc) if sub[i] > 0]
            if not nz:
                continue
            s0, s1 = nz[0] * KT, (nz[-1] + 1) * KT
            tiles.append(_KTile(kt, s0, s1, None))
        if not tiles:
            raise ValueError(f"query chunk {qc} has no unmasked keys")
        u0 = min(t.s0 for t in tiles)
        u1 = max(t.s1 for t in tiles)
        tiles[0].s0, tiles[0].s1 = u0, u1
        tiles[0].first = True
        tiles[-1].last = True
        for t in tiles:
            muls = []
            for qt in range(t.s0 // KT, t.s1 // KT):
                full = sums[t.kt, qc * qt_per_qc + qt]
                if full != KT * KT:          # zero or mixed -> needs pattern
                    muls.append((qt, pattern_id(t.kt, qc * qt_per_qc + qt)))
            t.muls = muls
        plan.append(tiles)
    pats = np.stack(patterns) if patterns else np.zeros((1, KT, KT), np.float16)
    return plan, pats


# ----- TileContext with a codegen-safe exit drain ---------------------------
# The stock kernel-tail drain carries one semaphore wait per engine/queue the
# kernel touched; CoreV3 codegen rejects instructions with more than two
# waits ("Too many sync wait commands").  Split the waits across preceding
# sync-engine nops, two per instruction, so the drain itself needs none.
class _TileContext(tile.TileContext):
    def _drain_and_barrier(self, tick_clock, wait_clock):
        from concourse.vector_clock import ScopedClock
        nc = self.nc
        probe = nc.sync.nop()
        wait_clock.add_sem_waits(
            probe.ins, ScopedClock({None: tick_clock.global_clock}))
        si = probe.ins.sync_info
        waits = list(si.on_wait) if si and si.on_wait else []
        if len(waits) > 1:
            probe.ins.sync_info = mybir.SyncInfo(
                on_wait=waits[:1], on_update=list(si.on_update or []))
            for w in waits[1:]:
                n = nc.sync.nop()
                n.ins.sync_info = mybir.SyncInfo(on_wait=[w], on_update=[])
        nc.sync.drain()
        nc.all_engine_barrier()
        assert self.sems is not None
        popped = nc._tile_sem_poison_stack.pop()
        assert popped is self._sem_poison
        nc.clear_and_free_semaphores(list(self.sems.allocated().values()))


# The same wait-count limit applies to ordinary engine instructions under
# this walrus build, so after the program is fully built, hoist all but one
# wait of every instruction onto preceding same-engine no-ops.
def _legalize_waits(nc, limit=1):
    for bb in nc.main_func.blocks:
        insts = list(bb.instructions)
        out = []
        for inst in insts:
            si = inst.sync_info
            waits = list(si.on_wait) if si and si.on_wait else []
            if len(waits) > limit:
                for w in waits[:-limit]:
                    nop = mybir.InstNoOp(
                        name=nc.get_next_instruction_name(), ins=[], outs=[])
                    nop.engine = inst.engine
                    nop.sync_info = mybir.SyncInfo(on_wait=[w], on_update=[])
                    nc.register_instruction(nop, overwrite=True)
                    out.append(nop)
                inst.sync_info = mybir.SyncInfo(
                    on_wait=waits[-limit:],
                    on_update=list(si.on_update or []))
            out.append(inst)
        bb.instructions = out


# ----- the bass program -----------------------------------------------------
def build_program(plan, n_pat):
    nc = bass.Bass(num_devices=NCORES)

    xq2 = nc.dram_tensor("xq2", [P, NQC * 8 * QC], F16, kind="ExternalInput")
    xk2 = nc.dram_tensor("xk2", [P, NQC * 8 * QC], F16, kind="ExternalInput")
    xv2 = nc.dram_tensor("xv2", [P, NQC * 8 * QC], F16, kind="ExternalInput")
    wqT = nc.dram_tensor("wqT", [D, EL], F16, kind="ExternalInput")
    wkT = nc.dram_tensor("wkT", [D, EL], F16, kind="ExternalInput")
    wvT = nc.dram_tensor("wvT", [D, EL], F16, kind="ExternalInput")
    woT = nc.dram_tensor("woT", [EL, D], F16, kind="ExternalInput")
    bq2 = nc.dram_tensor("bq2", [2, P], F32, kind="ExternalInput")
    pats = nc.dram_tensor("pats", [n_pat, KT, KT], F16, kind="ExternalInput")
    yO = nc.dram_tensor("yO", [NMT * NQC * P, QC], F16, kind="ExternalOutput")

    xrs = {
        "q": xq2.rearrange("p (c a t) -> p c a t", c=NQC, a=8),
        "k": xk2.rearrange("p (c a t) -> p c a t", c=NQC, a=8),
        "v": xv2.rearrange("p (c a t) -> p c a t", c=NQC, a=8),
    }

    with ExitStack() as ctx:
        tc = ctx.enter_context(_TileContext(nc))
        singles = ctx.enter_context(tc.tile_pool(name="singles", bufs=1))

        # --- persistent SBUF state ---
        wq_sb = singles.tile([P, 8, EL], F16)
        wk_sb = singles.tile([P, 8, EL], F16)
        wv_sb = singles.tile([P, 8, EL], F16)
        wo_sb = singles.tile([P, 2, D], F16)
        nc.sync.dma_start(out=wq_sb[:], in_=wqT.rearrange("(a p) e -> p a e", p=P))
        nc.sync.dma_start(out=wk_sb[:], in_=wkT.rearrange("(a p) e -> p a e", p=P))
        nc.sync.dma_start(out=wv_sb[:], in_=wvT.rearrange("(a p) e -> p a e", p=P))
        nc.sync.dma_start(out=wo_sb[:], in_=woT.rearrange("(a p) m -> p a m", p=P))
        bq_sb = singles.tile([P, 2], F32)
        nc.sync.dma_start(out=bq_sb[:], in_=bq2.rearrange("a p -> p a"))
        pat_sb = singles.tile([P, n_pat, KT], F16)
        nc.sync.dma_start(out=pat_sb[:], in_=pats.rearrange("n p k -> p n k"))

        Qt = singles.tile([P, 2, S], F16)     # [e-within-tile, e-tile, t]
        Kt = singles.tile([P, 2, S], F16)
        Vaug = singles.tile([P, NKT, HL, DK + 1], F16)  # [t-in-ktile, kt, h, e|1]
        nc.vector.memset(Vaug[:, :, :, DK:DK + 1], 1.0)
        ones_f32 = singles.tile([P, DK], F32)
        nc.vector.memset(ones_f32[:], 1.0)
        ones_col = singles.tile([P, 1], F16)
        nc.vector.memset(ones_col[:], 1.0)

        # warm the Exp activation table while the inputs stream in
        warms = singles.tile([1, 8], F32)
        nc.scalar.activation(out=warms[0:1, 0:1], in_=bq_sb[0:1, 0:1],
                             func=mybir.ActivationFunctionType.Exp)

        # --- pools ---
        xin = ctx.enter_context(tc.tile_pool(name="xin", bufs=2))
        spool = ctx.enter_context(tc.tile_pool(name="spool", bufs=2, space="PSUM"))
        avy = ctx.enter_context(tc.tile_pool(name="avy", bufs=2, space="PSUM"))
        gps = ctx.enter_context(tc.tile_pool(name="gps", bufs=2, space="PSUM"))
        ptp = ctx.enter_context(tc.tile_pool(name="ptp", bufs=4))
        xtp = ctx.enter_context(tc.tile_pool(name="xtp", bufs=3))
        nrm = ctx.enter_context(tc.tile_pool(name="nrm", bufs=4))
        ysb = ctx.enter_context(tc.tile_pool(name="ysb", bufs=3))
        dbp = ctx.enter_context(tc.tile_pool(name="dbp", bufs=2, space="DRAM"))

        # --- PE HAM warm-up: junk matmuls while the first chunk loads ---
        wrm = singles.tile([P, QC], F16)
        nc.vector.memset(wrm[:], 0.0)
        wps = gps.tile([P, QC], F32, tag="gp", name="warmps")
        for _ in range(10):
            nc.tensor.matmul(wps[:], lhsT=wrm[:, 0:P], rhs=wrm[:],
                             start=True, stop=True)

        # --- chunk input staging ---
        xch = {}

        def stage_dma(c, names=("q", "k", "v"), split=1, eng=None):
            chs = xch.setdefault(c, {})
            for name in names:
                t = xin.tile([P, 8, QC], F16, tag=f"x{name}", name=f"x_{name}{c}")
                w = 8 // split
                e = eng or nc.sync
                for s in range(split):
                    e.dma_start(out=t[:, s * w:(s + 1) * w, :],
                                in_=xrs[name][:, c, s * w:(s + 1) * w, :])
                chs[name] = t

        # --- projections for chunk c (Q, K, V) ---
        def emit_proj_qk(c):
            tsl = slice(c * QC, (c + 1) * QC)
            chs = xch[c]
            for which, w_sb, dst in (("q", wq_sb, Qt), ("k", wk_sb, Kt)):
                xt = chs.pop(which)
                for et in range(2):
                    ps = gps.tile([P, QC], F32, tag="gp", name=f"pj_{which}{c}{et}")
                    for ft in range(8):
                        nc.tensor.matmul(
                            ps[:],
                            lhsT=w_sb[:, ft, et * P:(et + 1) * P],
                            rhs=xt[:, ft, :],
                            start=(ft == 0), stop=(ft == 7))
                    if which == "q":
                        nc.vector.tensor_scalar_add(
                            out=dst[:, et, tsl], in0=ps[:],
                            scalar1=bq_sb[:, et:et + 1])
                    else:
                        nc.vector.tensor_copy(out=dst[:, et, tsl], in_=ps[:])

        def emit_proj_v(c):
            xt = xch[c].pop("v")
            for tt in range(4):
                ktg = c * 4 + tt
                ps = gps.tile([P, EL], F32, tag="gp", name=f"pj_v{ktg}")
                for ft in range(8):
                    nc.tensor.matmul(
                        ps[:],
                        lhsT=xt[:, ft, tt * P:(tt + 1) * P],
                        rhs=wv_sb[:, ft, :],
                        start=(ft == 0), stop=(ft == 7))
                nc.vector.tensor_copy(
                    out=Vaug[:, ktg, :, 0:DK],
                    in_=ps.rearrange("p (h e) -> p h e", h=HL)[:, :, :])

        def emit_proj(c):
            emit_proj_qk(c)
            emit_proj_v(c)

        def projw_unit(c, which, et):
            w_sb, dst = ((wq_sb, Qt) if which == "q" else (wk_sb, Kt))
            tsl = slice(c * QC, (c + 1) * QC)

            def f():
                xt = xch[c][which]
                ps = gps.tile([P, QC], F32, tag="gp", name=f"pu_{which}{c}{et}")
                for ft in range(8):
                    nc.tensor.matmul(
                        ps[:],
                        lhsT=w_sb[:, ft, et * P:(et + 1) * P],
                        rhs=xt[:, ft, :],
                        start=(ft == 0), stop=(ft == 7))
                if which == "q":
                    nc.vector.tensor_scalar_add(
                        out=dst[:, et, tsl], in0=ps[:],
                        scalar1=bq_sb[:, et:et + 1])
                else:
                    nc.vector.tensor_copy(out=dst[:, et, tsl], in_=ps[:])
            return f

        def projv_unit(c, tts):
            def f():
                xt = xch[c]["v"]
                for tt in tts:
                    ktg = c * 4 + tt
                    ps = gps.tile([P, EL], F32, tag="gp", name=f"pu_v{ktg}")
                    for ft in range(8):
                        nc.tensor.matmul(
                            ps[:],
                            lhsT=xt[:, ft, tt * P:(tt + 1) * P],
                            rhs=wv_sb[:, ft, :],
                            start=(ft == 0), stop=(ft == 7))
                    nc.vector.tensor_copy(
                        out=Vaug[:, ktg, :, 0:DK],
                        in_=ps.rearrange("p (h e) -> p h e", h=HL)[:, :, :])
            return f

        def yproj_unit(c, xTt, mts):
            def f():
                for mt in mts:
                    yp = gps.tile([P, QC], F32, tag="gp", name=f"yu{c}{mt}")
                    for ct in range(2):
                        nc.tensor.matmul(
                            yp[:],
                            lhsT=wo_sb[:, ct, mt * P:(mt + 1) * P],
                            rhs=xTt[:, ct, :],
                            start=(ct == 0), stop=(ct == 1))
                    ys = ysb.tile([P, QC], F16, tag="ys", name=f"ys{c}{mt}")
                    nc.vector.tensor_copy(out=ys[:], in_=yp[:])
                    nc.sync.dma_start(
                        out=yO[(mt * NQC + c) * P:(mt * NQC + c + 1) * P, :],
                        in_=ys[:])
            return f

        # --- output projection for chunk c ---
        def emit_yproj(c, xTt):
            for mt in range(NMT):
                yp = gps.tile([P, QC], F32, tag="gp", name=f"yp{c}{mt}")
                for ct in range(2):
                    nc.tensor.matmul(
                        yp[:],
                        lhsT=wo_sb[:, ct, mt * P:(mt + 1) * P],
                        rhs=xTt[:, ct, :],
                        start=(ct == 0), stop=(ct == 1))
                ys = ysb.tile([P, QC], F16, tag="ys", name=f"ys{c}{mt}")
                nc.vector.tensor_copy(out=ys[:], in_=yp[:])
                nc.sync.dma_start(
                    out=yO[(mt * NQC + c) * P:(mt * NQC + c + 1) * P, :],
                    in_=ys[:])

        # --- attention for chunk qc ---
        # softmax renormalize: xT_h = av[e] * recip(av[ones-row]).  Per head
        # pair, the two denominator rows are gathered (tiny DMAs) onto
        # partitions 0-1 of one tile, the reciprocal runs as one batched
        # Ln + Exp(-x) pair on ScalarE, and the recip rows bounce through
        # DRAM to partition-broadcast back to 64 lanes for the DVE multiply.
        def emit_attn(qc, fillers=None):
            tiles = plan[qc]
            xTt = xtp.tile([P, 2, QC], F16, tag="xT", name=f"xT{qc}")
            qsl = slice(qc * QC, (qc + 1) * QC)
            for hp in range(HL // 2):
                et = hp
                avs = [avy.tile([P, QC], F32, tag="avy",
                                name=f"av{qc}{2 * hp + hh}")
                       for hh in range(2)]
                ptbs = []

                def emit_av(ti, t):
                    for hh in range(2):
                        nc.tensor.matmul(
                            avs[hh][0:DK + 1, t.s0:t.s1],
                            lhsT=Vaug[:, t.kt, 2 * hp + hh, :],
                            rhs=ptbs[ti][:, hh, t.s0:t.s1],
                            start=t.first, stop=t.last,
                            skip_group_check=True)

                for ti, t in enumerate(tiles):
                    ps = spool.tile([P, 2 * QC], F32, tag="s",
                                    name=f"s{qc}{hp}{ti}")
                    for hh in range(2):
                        po = hh * DK
                        nc.tensor.matmul(
                            ps[:, hh * QC:(hh + 1) * QC],
                            lhsT=Kt[po:po + DK, et,
                                    t.kt * KT:(t.kt + 1) * KT],
                            rhs=Qt[po:po + DK, et, qsl],
                            start=True, stop=True)
                    if ti > 0:
                        emit_av(ti - 1, tiles[ti - 1])
                    ptb = ptp.tile([P, 2, QC], F16, tag="pt",
                                   name=f"pt{qc}{hp}{ti}")
                    ptbs.append(ptb)
                    nc.scalar.activation(
                        out=ptb.rearrange("p a c -> p (a c)")[:, :],
                        in_=ps[:],
                        func=mybir.ActivationFunctionType.Exp,
                        scale=float(SCALE))
                    for hh in range(2):
                        for qt, pid in t.muls:
                            sl = slice(qt * KT, (qt + 1) * KT)
                            nc.vector.tensor_tensor(
                                out=ptb[:, hh, sl],
                                in0=ptb[:, hh, sl],
                                in1=pat_sb[:, pid, :],
                                op=mybir.AluOpType.mult)
                    if fillers:
                        for f in fillers.pop((hp, ti), ()):
                            f()
                emit_av(len(tiles) - 1, tiles[-1])
                cps = []
                for hh in range(2):
                    cp = nrm.tile([P, QC], F32, tag="cp",
                                  name=f"cp{qc}{2 * hp + hh}")
                    nc.vector.tensor_copy(out=cp[0:DK + 1, :],
                                          in_=avs[hh][0:DK + 1, :])
                    cps.append(cp)
                if qc == NQC - 1 and hp == HL // 2 - 1:
                    # final pair: every DMA hop here is exposed span, and the
                    # idle gap would drop the PE to half clock for the last
                    # output projection.  Reciprocal rows stay on partition
                    # 64 and broadcast via a K=1 matmul (ones (x) recip_row)
                    # into a transient psum tile instead of bouncing through
                    # DRAM.
                    # keep the PE HAM clock warm through the normalize
                    # chain so the last output projection runs at full rate
                    wps2 = avy.tile([P, QC], F32, tag="avy", name="warmfin")
                    for _ in range(8):
                        nc.tensor.matmul(wps2[:], lhsT=wrm[:, 0:P],
                                         rhs=wrm[:], start=True, stop=True)
                    bcp = gps.tile([P, QC], F32, tag="gp", name="bcfin")
                    for hh in range(2):
                        rl = nrm.tile([P, QC], F32, tag="dn",
                                      name=f"rl{qc}{hh}")
                        rr = nrm.tile([P, QC], F32, tag="dr",
                                      name=f"rr{qc}{hh}")
                        nc.scalar.activation(
                            out=rl[DK:DK + 1, :], in_=cps[hh][DK:DK + 1, :],
                            func=mybir.ActivationFunctionType.Ln)
                        nc.scalar.activation(
                            out=rr[DK:DK + 1, :], in_=rl[DK:DK + 1, :],
                            func=mybir.ActivationFunctionType.Exp, scale=-1.0)
                        nc.tensor.matmul(
                            bcp[hh * DK:(hh + 1) * DK, :],
                            lhsT=ones_f32[DK:DK + 1, 0:DK],
                            rhs=rr[DK:DK + 1, :],
                            start=True, stop=True)
                        off = 0 if hh == 0 else DK
                        nc.vector.tensor_tensor(
                            out=xTt[off:off + DK, et, :],
                            in0=cps[hh][0:DK, :],
                            in1=bcp[hh * DK:(hh + 1) * DK, :],
                            op=mybir.AluOpType.mult)
                    return xTt
                dn2 = nrm.tile([2, QC], F32, tag="dn", name=f"dn{qc}{hp}")
                dr2 = nrm.tile([2, QC], F32, tag="dr", name=f"dr{qc}{hp}")
                for hh in range(2):
                    nc.sync.dma_start(out=dn2[hh:hh + 1, :],
                                      in_=cps[hh][DK:DK + 1, :])
                nc.scalar.activation(out=dr2[0:2, :], in_=dn2[0:2, :],
                                     func=mybir.ActivationFunctionType.Ln)
                nc.scalar.activation(out=dn2[0:2, :], in_=dr2[0:2, :],
                                     func=mybir.ActivationFunctionType.Exp,
                                     scale=-1.0)
                dnb = dbp.tile([2, QC], F32, tag="dnb", name=f"dnb{qc}{hp}")
                nc.sync.dma_start(out=dnb[:], in_=dn2[0:2, :])
                for hh in range(2):
                    bc = nrm.tile([P, QC], F32, tag="bc",
                                  name=f"bc{qc}{2 * hp + hh}")
                    nc.sync.dma_start(
                        out=bc[0:DK, :],
                        in_=dnb[hh:hh + 1, :].partition_broadcast(DK))
                    off = 0 if hh == 0 else DK
                    nc.vector.tensor_tensor(
                        out=xTt[off:off + DK, et, :], in0=cps[hh][0:DK, :],
                        in1=bc[0:DK, :], op=mybir.AluOpType.mult)
            return xTt

        # --- the pipelined schedule ---
        # chunk 0: q and k stream in (split, so projections start at the
        # half-way mark) before v competes for HBM bandwidth; scores of
        # chunk 0 feed ScalarE as early as possible.
        # The causal triangle makes late chunks exp-heavy (ScalarE-bound)
        # while early chunks leave the PE with surplus fill.  Defer the
        # movable tensor work -- yproj(1), yproj(2) and the K/V projections
        # of chunk 3 -- into the late-chunk gaps as interleaved fillers.
        stage_dma(0, names=("q",), split=4, eng=nc.scalar)
        stage_dma(0, names=("k",), split=4, eng=nc.scalar)
        emit_proj_qk(0)
        stage_dma(0, names=("v",), split=2, eng=nc.scalar)
        stage_dma(1)
        emit_proj_v(0)

        stage_dma(2)
        xT0 = emit_attn(0)
        emit_proj(1)
        emit_yproj(0, xT0)

        stage_dma(3)
        xT1 = emit_attn(1)
        emit_proj(2)

        xT2 = emit_attn(2, fillers={
            (0, 4): (yproj_unit(1, xT1, range(0, 4)),),
            (1, 4): (yproj_unit(1, xT1, range(4, 8)),),
        })
        projw_unit(3, "q", 0)()
        projw_unit(3, "q", 1)()

        xT3 = emit_attn(3, fillers={
            (0, 3): (projw_unit(3, "k", 0),),
            (0, 5): (projw_unit(3, "k", 1),),
            (0, 7): (projv_unit(3, (0, 1)),),
            (0, 9): (projv_unit(3, (2, 3)),),
            (1, 2): (yproj_unit(2, xT2, range(0, 4)),),
            (1, 6): (yproj_unit(2, xT2, range(4, 8)),),
        })
        emit_yproj(3, xT3)

    _legalize_waits(nc)
    # Raw Bass skips Bacc's codegen_inst_isa_subclasses pass; without it the
    # extended GpSimd instructions (partition_broadcast, the library reload)
    # reach walrus with empty .instr bytes -> "ISA wrong length".
    from concourse.library_overlay import lower_extended_insts
    lower_extended_insts(nc)
    return nc


# ----- SPMD runner ----------------------------------------------------------
# run_bass_kernel_spmd's axon path lowers through jax.jit(shard_map(...)),
# which this jax version emits as `call`-indirect HLO that the bass_exec
# compile hook rejects, and a single 8-replica launch isn't reachable from
# here.  Instead: one single-device jit per core (clean single-computation
# HLO), dispatched asynchronously on all 8 cores.  The NEFF is memoized by
# HLO bytes so walrus runs once, not 8 times.
_NEFF_MEMO = {}


def _install_ldw_opt():
    """walrus ships with --enable-ldw-opt=false hardcoded; without the pass
    every matmul serializes behind its LDWEIGHTS (~100ns per MM, ~65us over
    this kernel).  Rewrite the flag for our walrus invocations only."""
    from concourse import bass_utils
    inner = bass_utils.run_command
    if getattr(inner, "_is_ldw_hook", False):
        return

    def hook(cmd, *a, **kw):
        if (os.environ.get("BASS_LDW_OPT", "0") != "0"
                and any("walrus_driver" in str(c) for c in cmd[:1])):
            cmd = ["--enable-ldw-opt=true" if str(c) == "--enable-ldw-opt=false"
                   else c for c in cmd]
        return inner(cmd, *a, **kw)

    hook._is_ldw_hook = True
    bass_utils.run_command = hook


def _install_memo_hook():
    import libneuronxla
    from concourse.bass2jax import install_neuronx_cc_hook

    _install_ldw_opt()
    install_neuronx_cc_hook()
    inner = libneuronxla.neuronx_cc
    if getattr(inner, "_is_memo_hook", False):
        return

    def memo_hook(code, code_format, platform_version, file_prefix):
        import hashlib
        key = hashlib.sha256(bytes(code)).hexdigest()
        if key not in _NEFF_MEMO:
            _NEFF_MEMO[key] = inner(code, code_format, platform_version,
                                    file_prefix)
        return _NEFF_MEMO[key]

    memo_hook._is_memo_hook = True
    libneuronxla.neuronx_cc = memo_hook


def run_spmd(nc, in_maps):
    import jax
    from concourse.bass2jax import _bass_exec_p

    _install_memo_hook()
    n_cores = len(in_maps)
    partition_name = (nc.partition_id_tensor.name
                      if nc.partition_id_tensor is not None else None)
    in_names, out_names, out_avals = [], [], []
    for alloc in nc.m.functions[0].allocations:
        if not isinstance(alloc, mybir.MemoryLocationSet):
            continue
        name = alloc.memorylocations[0].name
        if alloc.kind == "ExternalInput":
            if name != partition_name:
                in_names.append(name)
        elif alloc.kind == "ExternalOutput":
            out_names.append(name)
            out_avals.append(jax.core.ShapedArray(
                tuple(alloc.tensor_shape), mybir.dt.np(alloc.dtype)))
    bind_in_names = tuple(in_names +
                          ([partition_name] if partition_name else []))

    def _body(*args):
        return tuple(_bass_exec_p.bind(
            *args, out_avals=tuple(out_avals), in_names=bind_in_names,
            out_names=tuple(out_names), lowering_input_output_aliases=(),
            sim_require_finite=True, sim_require_nnan=True, nc=nc))

    devices = jax.devices()[:n_cores]
    f = jax.jit(_body)
    futs = []
    for c in range(n_cores):
        args = [jax.device_put(np.asarray(in_maps[c][nm]), devices[c])
                for nm in in_names]
        if partition_name:
            args.append(jax.device_put(np.array([[c]], np.uint32), devices[c]))
        futs.append(f(*args))
    return [{nm: np.asarray(futs[c][i]) for i, nm in enumerate(out_names)}
            for c in range(n_cores)]


# ----- host wrapper ---------------------------------------------------------
_CACHE = {}


def _get_program(mask):
    key = mask.tobytes()
    if key not in _CACHE:
        plan, pats = _mask_plan(mask)
        nc = build_program(plan, pats.shape[0])
        _CACHE[key] = (nc, pats)
    return _CACHE[key]


def _chunk_major(xT):
    """[D, S] f32 -> [P, NQC*8*QC] f16 chunk-major for contiguous chunk DMA"""
    return np.ascontiguousarray(
        xT.astype(F16NP).reshape(8, P, NQC, QC).transpose(1, 2, 0, 3)
        .reshape(P, NQC * 8 * QC))


def make_in_maps(q, k, v, mask, wq, bq, wk, bk, wv, bv, wo, bo, pats):
    q, k, v = (np.asarray(a, np.float32) for a in (q, k, v))
    in_maps = []
    for c in range(NCORES):
        b, g = divmod(c, TP)
        sl = slice(g * EL, (g + 1) * EL)
        in_maps.append({
            "xq2": _chunk_major(q[b].T),
            "xk2": _chunk_major(k[b].T),
            "xv2": _chunk_major(v[b].T),
            "wqT": np.ascontiguousarray(wq[sl, :].T.astype(F16NP)),
            "wkT": np.ascontiguousarray(wk[sl, :].T.astype(F16NP)),
            "wvT": np.ascontiguousarray(wv[sl, :].T.astype(F16NP)),
            "woT": np.ascontiguousarray(wo[:, sl].T).astype(F16NP),
            "bq2": np.ascontiguousarray(bq[sl].reshape(2, P)),
            "pats": pats,
        })
    return in_maps


def assemble_output(results, bv, wo, bo):
    extra = (np.asarray(bv, np.float64) @ np.asarray(wo, np.float64).T
             + np.asarray(bo, np.float64)).astype(np.float32)  # [D]
    y = np.empty((B, S, D), np.float32)
    for b in range(B):
        acc = np.zeros((D, S), np.float32)
        for g in range(TP):
            yT = (results[b * TP + g]["yO"].astype(np.float32)
                  .reshape(NMT, NQC, P, QC).transpose(0, 2, 1, 3)
                  .reshape(D, S))
            acc += yT
        y[b] = acc.T + extra[None, :]
    return y


def kernel(q, k, v, mask, wq, bq, wk, bk, wv, bv, wo, bo):
    mask2d = np.asarray(mask).reshape(S, S)
    nc, pats = _get_program(mask2d)
    in_maps = make_in_maps(q, k, v, mask2d, wq, bq, wk, bk, wv, bv, wo, bo, pats)
    return assemble_output(run_spmd(nc, in_maps), bv, wo, bo)


# revision 28
# speedup vs baseline: 1.1047x; 1.0020x over previous
"""Multi-head attention (B=2, S=2048, D=1024, H=16, causal mask) on 8 trn2
NeuronCores.

Sharding: 2-way data parallel over batch x 4-way tensor parallel over head
groups (4 heads / core).  Core c handles batch c//4, head group c%4.

Everything on-chip lives feature-major ("transposed") so no transposes are
ever needed: inputs arrive chunk-major [P, chunk, ftile, t] in fp16; Q/K
projections produce Qh^T/Kh^T [e, t]; scores come out keys-major [k, q];
exp(p) feeds A@V directly as the moving operand with V (+ a ones column
that makes the softmax denominator fall out of the same matmul)
stationary; the attention output appears as x_att^T [e, q], which is
exactly the layout the output projection wants.  Each core emits its y^T
partial and the host sums the 4 partials of each batch group (row-parallel
tensor-parallel reduction).

Differences vs the phase-separated v1:
  * single software-pipelined loop over query chunks: the projection
    matmuls of chunk c+1 fill the tensor-engine gaps left while ScalarE
    runs the exp stream of chunk c (the Tile scheduler interleaves by
    priority/readiness; ScalarE exp is the critical resource at ~92us).
  * K bias dropped entirely (adding bk shifts every score column by a
    per-query constant, which softmax cancels exactly).
  * V bias and output bias are applied analytically on the host
    (bv @ wo.T + bo added once after the partial-sum gather).
  * softmax denominator reciprocal as a batched 2-row Ln + Exp(-x) pair
    on ScalarE per head pair; the recip rows partition-broadcast through
    a DRAM bounce mid-kernel, and via a K=1 ones matmul on the (otherwise
    idle) PE for the final pair so the last output projection is not
    delayed and stays on a warm clock.
  * exp always runs full-width [128, 2*QC] (one ACTIVATE per k-tile per
    head pair); scores matmuls write the full query range so the psum is
    fully initialized.  Mask patterns zero the dead blocks after exp.
  * junk matmuls warm the PE HAM clock gate while the first input chunk
    streams in (and again under the final normalize chain).
  * y partials stream out as fp16 directly from a psum->sbuf cast.

Mask handled generically: the [S,S] mask is classified on the host into
128x128 blocks (zero / one / mixed).  Zero k-tiles are skipped entirely,
mixed blocks get a pattern-multiply after exp with deduplicated patterns
uploaded as data.
"""

import os
import sys

import numpy as np

for _p in ("/opt/trn_rl_repo", "/root/.axon_site/_ro/trn_rl_repo"):
    if os.path.isdir(_p) and _p not in sys.path:
        sys.path.append(_p)

import ml_dtypes  # noqa: E402
from contextlib import ExitStack  # noqa: E402

import concourse.bass as bass  # noqa: E402
import concourse.tile as tile  # noqa: E402
from concourse import mybir  # noqa: E402

# ----- problem constants (hardcoded per contract) ---------------------------
B, S, D, H, DK = 2, 2048, 1024, 16, 64
NCORES = 8
TP = 4                      # head-parallel ways (per batch group)
EL = D // TP                # 256 local head dims = 4 heads
HL = H // TP                # 4 local heads
QC = 512                    # query-chunk (columns per attention pass)
NQC = S // QC               # 4
KT = 128                    # key tile (contraction tile for A@V)
NKT = S // KT               # 16
P = 128
NMT = D // P                # 8 output-feature tiles
SCALE = 1.0 / np.sqrt(DK)

F32 = mybir.dt.float32
F16 = mybir.dt.float16
F16NP = np.float16


# ----- host-side mask analysis ---------------------------------------------
class _KTile:
    __slots__ = ("kt", "s0", "s1", "muls", "first", "last")

    def __init__(self, kt, s0, s1, muls):
        self.kt, self.s0, self.s1, self.muls = kt, s0, s1, muls
        self.first = False
        self.last = False


def _mask_plan(mask2d):
    """mask2d: [S, S] ints, mask2d[q, k] (1 = attend).  Returns
    (plan, patterns) where plan[qc] is a list of _KTile and patterns is a
    f16 array [n_pat, 128, 128] of transposed (k-major) mask blocks."""
    mT = (mask2d != 0).astype(np.float32).T          # [k, q]
    nqt = S // KT
    blk = mT.reshape(NKT, KT, nqt, KT).transpose(0, 2, 1, 3)  # [kt, qt, 128, 128]
    sums = blk.sum(axis=(2, 3))
    patterns = []
    pat_idx = {}

    def pattern_id(kt, qt):
        key = blk[kt, qt].tobytes()
        if key not in pat_idx:
            pat_idx[key] = len(patterns)
            patterns.append(blk[kt, qt].astype(np.float16))
        return pat_idx[key]

    qt_per_qc = QC // KT
    plan = []
    for qc in range(NQC):
        tiles = []
        for kt in range(NKT):
            sub = sums[kt, qc * qt_per_qc:(qc + 1) * qt_per_qc]
            nz = [i for i in range(qt_per_qc) if sub[i] > 0]
            if not nz:
                continue
            s0, s1 = nz[0] * KT, (nz[-1] + 1) * KT
            tiles.append(_KTile(kt, s0, s1, None))
        if not tiles:
            raise ValueError(f"query chunk {qc} has no unmasked keys")
        u0 = min(t.s0 for t in tiles)
        u1 = max(t.s1 for t in tiles)
        tiles[0].s0, tiles[0].s1 = u0, u1
        tiles[0].first = True
        tiles[-1].last = True
        for t in tiles:
            muls = []
            for qt in range(t.s0 // KT, t.s1 // KT):
                full = sums[t.kt, qc * qt_per_qc + qt]
                if full != KT * KT:          # zero or mixed -> needs pattern
                    muls.append((qt, pattern_id(t.kt, qc * qt_per_qc + qt)))
            t.muls = muls
        plan.append(tiles)
    pats = np.stack(patterns) if patterns else np.zeros((1, KT, KT), np.float16)
    return plan, pats


# ----- TileContext with a codegen-safe exit drain ---------------------------
# The stock kernel-tail drain carries one semaphore wait per engine/queue the
# kernel touched; CoreV3 codegen rejects instructions with more than two
# waits ("Too many sync wait commands").  Split the waits across preceding
# sync-engine nops, two per instruction, so the drain itself needs none.
class _TileContext(tile.TileContext):
    def _drain_and_barrier(self, tick_clock, wait_clock):
        from concourse.vector_clock import ScopedClock
        nc = self.nc
        probe = nc.sync.nop()
        wait_clock.add_sem_waits(
            probe.ins, ScopedClock({None: tick_clock.global_clock}))
        si = probe.ins.sync_info
        waits = list(si.on_wait) if si and si.on_wait else []
        if len(waits) > 1:
            probe.ins.sync_info = mybir.SyncInfo(
                on_wait=waits[:1], on_update=list(si.on_update or []))
            for w in waits[1:]:
                n = nc.sync.nop()
                n.ins.sync_info = mybir.SyncInfo(on_wait=[w], on_update=[])
        nc.sync.drain()
        nc.all_engine_barrier()
        assert self.sems is not None
        popped = nc._tile_sem_poison_stack.pop()
        assert popped is self._sem_poison
        nc.clear_and_free_semaphores(list(self.sems.allocated().values()))


# The same wait-count limit applies to ordinary engine instructions under
# this walrus build, so after the program is fully built, hoist all but one
# wait of every instruction onto preceding same-engine no-ops.
def _legalize_waits(nc, limit=1):
    for bb in nc.main_func.blocks:
        insts = list(bb.instructions)
        out = []
        for inst in insts:
            si = inst.sync_info
            waits = list(si.on_wait) if si and si.on_wait else []
            if len(waits) > limit:
                for w in waits[:-limit]:
                    nop = mybir.InstNoOp(
                        name=nc.get_next_instruction_name(), ins=[], outs=[])
                    nop.engine = inst.engine
                    nop.sync_info = mybir.SyncInfo(on_wait=[w], on_update=[])
                    nc.register_instruction(nop, overwrite=True)
                    out.append(nop)
                inst.sync_info = mybir.SyncInfo(
                    on_wait=waits[-limit:],
                    on_update=list(si.on_update or []))
            out.append(inst)
        bb.instructions = out


# ----- the bass program -----------------------------------------------------
def build_program(plan, n_pat):
    nc = bass.Bass(num_devices=NCORES)

    xq2 = nc.dram_tensor("xq2", [P, NQC * 8 * QC], F16, kind="ExternalInput")
    xk2 = nc.dram_tensor("xk2", [P, NQC * 8 * QC], F16, kind="ExternalInput")
    xv2 = nc.dram_tensor("xv2", [P, NQC * 8 * QC], F16, kind="ExternalInput")
    wqT = nc.dram_tensor("wqT", [D, EL], F16, kind="ExternalInput")
    wkT = nc.dram_tensor("wkT", [D, EL], F16, kind="ExternalInput")
    wvT = nc.dram_tensor("wvT", [D, EL], F16, kind="ExternalInput")
    woT = nc.dram_tensor("woT", [EL, D], F16, kind="ExternalInput")
    bq2 = nc.dram_tensor("bq2", [2, P], F32, kind="ExternalInput")
    pats = nc.dram_tensor("pats", [n_pat, KT, KT], F16, kind="ExternalInput")
    yO = nc.dram_tensor("yO", [NMT * NQC * P, QC], F16, kind="ExternalOutput")

    xrs = {
        "q": xq2.rearrange("p (c a t) -> p c a t", c=NQC, a=8),
        "k": xk2.rearrange("p (c a t) -> p c a t", c=NQC, a=8),
        "v": xv2.rearrange("p (c a t) -> p c a t", c=NQC, a=8),
    }

    with ExitStack() as ctx:
        tc = ctx.enter_context(_TileContext(nc))
        singles = ctx.enter_context(tc.tile_pool(name="singles", bufs=1))

        # --- persistent SBUF state ---
        wq_sb = singles.tile([P, 8, EL], F16)
        wk_sb = singles.tile([P, 8, EL], F16)
        wv_sb = singles.tile([P, 8, EL], F16)
        wo_sb = singles.tile([P, 2, D], F16)
        nc.sync.dma_start(out=wq_sb[:], in_=wqT.rearrange("(a p) e -> p a e", p=P))
        nc.sync.dma_start(out=wk_sb[:], in_=wkT.rearrange("(a p) e -> p a e", p=P))
        nc.sync.dma_start(out=wv_sb[:], in_=wvT.rearrange("(a p) e -> p a e", p=P))
        nc.sync.dma_start(out=wo_sb[:], in_=woT.rearrange("(a p) m -> p a m", p=P))
        bq_sb = singles.tile([P, 2], F32)
        nc.sync.dma_start(out=bq_sb[:], in_=bq2.rearrange("a p -> p a"))
        pat_sb = singles.tile([P, n_pat, KT], F16)
        nc.sync.dma_start(out=pat_sb[:], in_=pats.rearrange("n p k -> p n k"))

        Qt = singles.tile([P, 2, S], F16)     # [e-within-tile, e-tile, t]
        Kt = singles.tile([P, 2, S], F16)
        Vaug = singles.tile([P, NKT, HL, DK + 1], F16)  # [t-in-ktile, kt, h, e|1]
        nc.vector.memset(Vaug[:, :, :, DK:DK + 1], 1.0)
        ones_f32 = singles.tile([P, DK], F32)
        nc.vector.memset(ones_f32[:], 1.0)
        ones_col = singles.tile([P, 1], F16)
        nc.vector.memset(ones_col[:], 1.0)

        # warm the Exp activation table while the inputs stream in
        warms = singles.tile([1, 8], F32)
        nc.scalar.activation(out=warms[0:1, 0:1], in_=bq_sb[0:1, 0:1],
                             func=mybir.ActivationFunctionType.Exp)

        # --- pools ---
        xin = ctx.enter_context(tc.tile_pool(name="xin", bufs=2))
        spool = ctx.enter_context(tc.tile_pool(name="spool", bufs=2, space="PSUM"))
        avy = ctx.enter_context(tc.tile_pool(name="avy", bufs=2, space="PSUM"))
        gps = ctx.enter_context(tc.tile_pool(name="gps", bufs=2, space="PSUM"))
        ptp = ctx.enter_context(tc.tile_pool(name="ptp", bufs=4))
        xtp = ctx.enter_context(tc.tile_pool(name="xtp", bufs=3))
        nrm = ctx.enter_context(tc.tile_pool(name="nrm", bufs=4))
        ysb = ctx.enter_context(tc.tile_pool(name="ysb", bufs=3))
        dbp = ctx.enter_context(tc.tile_pool(name="dbp", bufs=2, space="DRAM"))

        # --- PE HAM warm-up: junk matmuls while the first chunk loads ---
        wrm = singles.tile([P, QC], F16)
        nc.vector.memset(wrm[:], 0.0)
        wps = gps.tile([P, QC], F32, tag="gp", name="warmps")
        for _ in range(18):
            nc.tensor.matmul(wps[:], lhsT=wrm[:, 0:P], rhs=wrm[:],
                             start=True, stop=True)

        # --- chunk input staging ---
        xch = {}

        def stage_dma(c, names=("q", "k", "v"), split=1, eng=None):
            chs = xch.setdefault(c, {})
            for name in names:
                t = xin.tile([P, 8, QC], F16, tag=f"x{name}", name=f"x_{name}{c}")
                w = 8 // split
                e = eng or nc.sync
                for s in range(split):
                    e.dma_start(out=t[:, s * w:(s + 1) * w, :],
                                in_=xrs[name][:, c, s * w:(s + 1) * w, :])
                chs[name] = t

        # --- projections for chunk c (Q, K, V) ---
        def emit_proj_qk(c):
            tsl = slice(c * QC, (c + 1) * QC)
            chs = xch[c]
            for which, w_sb, dst in (("q", wq_sb, Qt), ("k", wk_sb, Kt)):
                xt = chs.pop(which)
                for et in range(2):
                    ps = gps.tile([P, QC], F32, tag="gp", name=f"pj_{which}{c}{et}")
                    for ft in range(8):
                        nc.tensor.matmul(
                            ps[:],
                            lhsT=w_sb[:, ft, et * P:(et + 1) * P],
                            rhs=xt[:, ft, :],
                            start=(ft == 0), stop=(ft == 7))
                    if which == "q":
                        nc.vector.tensor_scalar_add(
                            out=dst[:, et, tsl], in0=ps[:],
                            scalar1=bq_sb[:, et:et + 1])
                    else:
                        nc.vector.tensor_copy(out=dst[:, et, tsl], in_=ps[:])

        def emit_proj_v(c):
            xt = xch[c].pop("v")
            for tt in range(4):
                ktg = c * 4 + tt
                ps = gps.tile([P, EL], F32, tag="gp", name=f"pj_v{ktg}")
                for ft in range(8):
                    nc.tensor.matmul(
                        ps[:],
                        lhsT=xt[:, ft, tt * P:(tt + 1) * P],
                        rhs=wv_sb[:, ft, :],
                        start=(ft == 0), stop=(ft == 7))
                nc.vector.tensor_copy(
                    out=Vaug[:, ktg, :, 0:DK],
                    in_=ps.rearrange("p (h e) -> p h e", h=HL)[:, :, :])

        def emit_proj(c):
            emit_proj_qk(c)
            emit_proj_v(c)

        def projw_unit(c, which, et):
            w_sb, dst = ((wq_sb, Qt) if which == "q" else (wk_sb, Kt))
            tsl = slice(c * QC, (c + 1) * QC)

            def f():
                xt = xch[c][which]
                ps = gps.tile([P, QC], F32, tag="gp", name=f"pu_{which}{c}{et}")
                for ft in range(8):
                    nc.tensor.matmul(
                        ps[:],
                        lhsT=w_sb[:, ft, et * P:(et + 1) * P],
                        rhs=xt[:, ft, :],
                        start=(ft == 0), stop=(ft == 7))
                if which == "q":
                    nc.vector.tensor_scalar_add(
                        out=dst[:, et, tsl], in0=ps[:],
                        scalar1=bq_sb[:, et:et + 1])
                else:
                    nc.vector.tensor_copy(out=dst[:, et, tsl], in_=ps[:])
            return f

        def projv_unit(c, tts):
            def f():
                xt = xch[c]["v"]
                for tt in tts:
                    ktg = c * 4 + tt
                    ps = gps.tile([P, EL], F32, tag="gp", name=f"pu_v{ktg}")
                    for ft in range(8):
                        nc.tensor.matmul(
                            ps[:],
                            lhsT=xt[:, ft, tt * P:(tt + 1) * P],
                            rhs=wv_sb[:, ft, :],
                            start=(ft == 0), stop=(ft == 7))
                    nc.vector.tensor_copy(
                        out=Vaug[:, ktg, :, 0:DK],
                        in_=ps.rearrange("p (h e) -> p h e", h=HL)[:, :, :])
            return f

        def yproj_unit(c, xTt, mts):
            def f():
                for mt in mts:
                    yp = gps.tile([P, QC], F32, tag="gp", name=f"yu{c}{mt}")
                    for ct in range(2):
                        nc.tensor.matmul(
                            yp[:],
                            lhsT=wo_sb[:, ct, mt * P:(mt + 1) * P],
                            rhs=xTt[:, ct, :],
                            start=(ct == 0), stop=(ct == 1))
                    ys = ysb.tile([P, QC], F16, tag="ys", name=f"ys{c}{mt}")
                    nc.vector.tensor_copy(out=ys[:], in_=yp[:])
                    nc.sync.dma_start(
                        out=yO[(mt * NQC + c) * P:(mt * NQC + c + 1) * P, :],
                        in_=ys[:])
            return f

        # --- output projection for chunk c ---
        def emit_yproj(c, xTt):
            for mt in range(NMT):
                yp = gps.tile([P, QC], F32, tag="gp", name=f"yp{c}{mt}")
                for ct in range(2):
                    nc.tensor.matmul(
                        yp[:],
                        lhsT=wo_sb[:, ct, mt * P:(mt + 1) * P],
                        rhs=xTt[:, ct, :],
                        start=(ct == 0), stop=(ct == 1))
                ys = ysb.tile([P, QC], F16, tag="ys", name=f"ys{c}{mt}")
                nc.vector.tensor_copy(out=ys[:], in_=yp[:])
                nc.sync.dma_start(
                    out=yO[(mt * NQC + c) * P:(mt * NQC + c + 1) * P, :],
                    in_=ys[:])

        # --- attention for chunk qc ---
        # softmax renormalize: xT_h = av[e] * recip(av[ones-row]).  Per head
        # pair, the two denominator rows are gathered (tiny DMAs) onto
        # partitions 0-1 of one tile, the reciprocal runs as one batched
        # Ln + Exp(-x) pair on ScalarE, and the recip rows bounce through
        # DRAM to partition-broadcast back to 64 lanes for the DVE multiply.
        def emit_attn(qc, fillers=None):
            tiles = plan[qc]
            xTt = xtp.tile([P, 2, QC], F16, tag="xT", name=f"xT{qc}")
            qsl = slice(qc * QC, (qc + 1) * QC)
            for hp in range(HL // 2):
                et = hp
                avs = [avy.tile([P, QC], F32, tag="avy",
                                name=f"av{qc}{2 * hp + hh}")
                       for hh in range(2)]
                ptbs = []

                def emit_av(ti, t):
                    for hh in range(2):
                        nc.tensor.matmul(
                            avs[hh][0:DK + 1, t.s0:t.s1],
                            lhsT=Vaug[:, t.kt, 2 * hp + hh, :],
                            rhs=ptbs[ti][:, hh, t.s0:t.s1],
                            start=t.first, stop=t.last,
                            skip_group_check=True)

                for ti, t in enumerate(tiles):
                    ps = spool.tile([P, 2 * QC], F32, tag="s",
                                    name=f"s{qc}{hp}{ti}")
                    for hh in range(2):
                        po = hh * DK
                        nc.tensor.matmul(
                            ps[:, hh * QC:(hh + 1) * QC],
                            lhsT=Kt[po:po + DK, et,
                                    t.kt * KT:(t.kt + 1) * KT],
                            rhs=Qt[po:po + DK, et, qsl],
                            start=True, stop=True)
                    if ti > 0:
                        emit_av(ti - 1, tiles[ti - 1])
                    ptb = ptp.tile([P, 2, QC], F16, tag="pt",
                                   name=f"pt{qc}{hp}{ti}")
                    ptbs.append(ptb)
                    nc.scalar.activation(
                        out=ptb.rearrange("p a c -> p (a c)")[:, :],
                        in_=ps[:],
                        func=mybir.ActivationFunctionType.Exp,
                        scale=float(SCALE))
                    for hh in range(2):
                        for qt, pid in t.muls:
                            sl = slice(qt * KT, (qt + 1) * KT)
                            nc.vector.tensor_tensor(
                                out=ptb[:, hh, sl],
                                in0=ptb[:, hh, sl],
                                in1=pat_sb[:, pid, :],
                                op=mybir.AluOpType.mult)
                    if fillers:
                        for f in fillers.pop((hp, ti), ()):
                            f()
                emit_av(len(tiles) - 1, tiles[-1])
                cps = []
                for hh in range(2):
                    cp = nrm.tile([P, QC], F32, tag="cp",
                                  name=f"cp{qc}{2 * hp + hh}")
                    nc.vector.tensor_copy(out=cp[0:DK + 1, :],
                                          in_=avs[hh][0:DK + 1, :])
                    cps.append(cp)
                if qc == NQC - 1 and hp == HL // 2 - 1:
                    # final pair: every DMA hop here is exposed span, and the
                    # idle gap would drop the PE to half clock for the last
                    # output projection.  Reciprocal rows stay on partition
                    # 64 and broadcast via a K=1 matmul (ones (x) recip_row)
                    # into a transient psum tile instead of bouncing through
                    # DRAM.
                    # keep the PE HAM clock warm through the normalize
                    # chain so the last output projection runs at full rate
                    wps2 = avy.tile([P, QC], F32, tag="avy", name="warmfin")
                    for _ in range(8):
                        nc.tensor.matmul(wps2[:], lhsT=wrm[:, 0:P],
                                         rhs=wrm[:], start=True, stop=True)
                    bcp = gps.tile([P, QC], F32, tag="gp", name="bcfin")
                    for hh in range(2):
                        rl = nrm.tile([P, QC], F32, tag="dn",
                                      name=f"rl{qc}{hh}")
                        rr = nrm.tile([P, QC], F32, tag="dr",
                                      name=f"rr{qc}{hh}")
                        nc.scalar.activation(
                            out=rl[DK:DK + 1, :], in_=cps[hh][DK:DK + 1, :],
                            func=mybir.ActivationFunctionType.Ln)
                        nc.scalar.activation(
                            out=rr[DK:DK + 1, :], in_=rl[DK:DK + 1, :],
                            func=mybir.ActivationFunctionType.Exp, scale=-1.0)
                        nc.tensor.matmul(
                            bcp[hh * DK:(hh + 1) * DK, :],
                            lhsT=ones_f32[DK:DK + 1, 0:DK],
                            rhs=rr[DK:DK + 1, :],
                            start=True, stop=True)
                        off = 0 if hh == 0 else DK
                        nc.vector.tensor_tensor(
                            out=xTt[off:off + DK, et, :],
                            in0=cps[hh][0:DK, :],
                            in1=bcp[hh * DK:(hh + 1) * DK, :],
                            op=mybir.AluOpType.mult)
                    return xTt
                dn2 = nrm.tile([2, QC], F32, tag="dn", name=f"dn{qc}{hp}")
                dr2 = nrm.tile([2, QC], F32, tag="dr", name=f"dr{qc}{hp}")
                for hh in range(2):
                    nc.sync.dma_start(out=dn2[hh:hh + 1, :],
                                      in_=cps[hh][DK:DK + 1, :])
                nc.scalar.activation(out=dr2[0:2, :], in_=dn2[0:2, :],
                                     func=mybir.ActivationFunctionType.Ln)
                nc.scalar.activation(out=dn2[0:2, :], in_=dr2[0:2, :],
                                     func=mybir.ActivationFunctionType.Exp,
                                     scale=-1.0)
                dnb = dbp.tile([2, QC], F32, tag="dnb", name=f"dnb{qc}{hp}")
                nc.sync.dma_start(out=dnb[:], in_=dn2[0:2, :])
                for hh in range(2):
                    bc = nrm.tile([P, QC], F32, tag="bc",
                                  name=f"bc{qc}{2 * hp + hh}")
                    nc.sync.dma_start(
                        out=bc[0:DK, :],
                        in_=dnb[hh:hh + 1, :].partition_broadcast(DK))
                    off = 0 if hh == 0 else DK
                    nc.vector.tensor_tensor(
                        out=xTt[off:off + DK, et, :], in0=cps[hh][0:DK, :],
                        in1=bc[0:DK, :], op=mybir.AluOpType.mult)
            return xTt

        # --- the pipelined schedule ---
        # chunk 0: q and k stream in (split, so projections start at the
        # half-way mark) before v competes for HBM bandwidth; scores of
        # chunk 0 feed ScalarE as early as possible.
        # The causal triangle makes late chunks exp-heavy (ScalarE-bound)
        # while early chunks leave the PE with surplus fill.  Defer the
        # movable tensor work -- yproj(1), yproj(2) and the K/V projections
        # of chunk 3 -- into the late-chunk gaps as interleaved fillers.
        stage_dma(0, names=("q",), split=4, eng=nc.scalar)
        stage_dma(0, names=("k",), split=4, eng=nc.scalar)
        emit_proj_qk(0)
        stage_dma(0, names=("v",), split=2, eng=nc.scalar)
        stage_dma(1)
        emit_proj_v(0)

        stage_dma(2)
        xT0 = emit_attn(0)
        emit_proj(1)
        emit_yproj(0, xT0)

        stage_dma(3)
        xT1 = emit_attn(1)
        emit_proj(2)

        xT2 = emit_attn(2, fillers={
            (0, 4): (yproj_unit(1, xT1, range(0, 4)),),
            (1, 4): (yproj_unit(1, xT1, range(4, 8)),),
        })
        projw_unit(3, "q", 0)()
        projw_unit(3, "q", 1)()

        xT3 = emit_attn(3, fillers={
            (0, 3): (projw_unit(3, "k", 0),),
            (0, 5): (projw_unit(3, "k", 1),),
            (0, 7): (projv_unit(3, (0, 1)),),
            (0, 9): (projv_unit(3, (2, 3)),),
            (1, 2): (yproj_unit(2, xT2, range(0, 4)),),
            (1, 6): (yproj_unit(2, xT2, range(4, 8)),),
        })
        emit_yproj(3, xT3)

    _legalize_waits(nc)
    # Raw Bass skips Bacc's codegen_inst_isa_subclasses pass; without it the
    # extended GpSimd instructions (partition_broadcast, the library reload)
    # reach walrus with empty .instr bytes -> "ISA wrong length".
    from concourse.library_overlay import lower_extended_insts
    lower_extended_insts(nc)
    return nc


# ----- SPMD runner ----------------------------------------------------------
# run_bass_kernel_spmd's axon path lowers through jax.jit(shard_map(...)),
# which this jax version emits as `call`-indirect HLO that the bass_exec
# compile hook rejects, and a single 8-replica launch isn't reachable from
# here.  Instead: one single-device jit per core (clean single-computation
# HLO), dispatched asynchronously on all 8 cores.  The NEFF is memoized by
# HLO bytes so walrus runs once, not 8 times.
_NEFF_MEMO = {}


def _install_ldw_opt():
    """walrus ships with --enable-ldw-opt=false hardcoded; without the pass
    every matmul serializes behind its LDWEIGHTS (~100ns per MM, ~65us over
    this kernel).  Rewrite the flag for our walrus invocations only."""
    from concourse import bass_utils
    inner = bass_utils.run_command
    if getattr(inner, "_is_ldw_hook", False):
        return

    def hook(cmd, *a, **kw):
        if (os.environ.get("BASS_LDW_OPT", "0") != "0"
                and any("walrus_driver" in str(c) for c in cmd[:1])):
            cmd = ["--enable-ldw-opt=true" if str(c) == "--enable-ldw-opt=false"
                   else c for c in cmd]
        return inner(cmd, *a, **kw)

    hook._is_ldw_hook = True
    bass_utils.run_command = hook


def _install_memo_hook():
    import libneuronxla
    from concourse.bass2jax import install_neuronx_cc_hook

    _install_ldw_opt()
    install_neuronx_cc_hook()
    inner = libneuronxla.neuronx_cc
    if getattr(inner, "_is_memo_hook", False):
        return

    def memo_hook(code, code_format, platform_version, file_prefix):
        import hashlib
        key = hashlib.sha256(bytes(code)).hexdigest()
        if key not in _NEFF_MEMO:
            _NEFF_MEMO[key] = inner(code, code_format, platform_version,
                                    file_prefix)
        return _NEFF_MEMO[key]

    memo_hook._is_memo_hook = True
    libneuronxla.neuronx_cc = memo_hook


def run_spmd(nc, in_maps):
    import jax
    from concourse.bass2jax import _bass_exec_p

    _install_memo_hook()
    n_cores = len(in_maps)
    partition_name = (nc.partition_id_tensor.name
                      if nc.partition_id_tensor is not None else None)
    in_names, out_names, out_avals = [], [], []
    for alloc in nc.m.functions[0].allocations:
        if not isinstance(alloc, mybir.MemoryLocationSet):
            continue
        name = alloc.memorylocations[0].name
        if alloc.kind == "ExternalInput":
            if name != partition_name:
                in_names.append(name)
        elif alloc.kind == "ExternalOutput":
            out_names.append(name)
            out_avals.append(jax.core.ShapedArray(
                tuple(alloc.tensor_shape), mybir.dt.np(alloc.dtype)))
    bind_in_names = tuple(in_names +
                          ([partition_name] if partition_name else []))

    def _body(*args):
        return tuple(_bass_exec_p.bind(
            *args, out_avals=tuple(out_avals), in_names=bind_in_names,
            out_names=tuple(out_names), lowering_input_output_aliases=(),
            sim_require_finite=True, sim_require_nnan=True, nc=nc))

    devices = jax.devices()[:n_cores]
    f = jax.jit(_body)
    futs = []
    for c in range(n_cores):
        args = [jax.device_put(np.asarray(in_maps[c][nm]), devices[c])
                for nm in in_names]
        if partition_name:
            args.append(jax.device_put(np.array([[c]], np.uint32), devices[c]))
        futs.append(f(*args))
    return [{nm: np.asarray(futs[c][i]) for i, nm in enumerate(out_names)}
            for c in range(n_cores)]


# ----- host wrapper ---------------------------------------------------------
_CACHE = {}


def _get_program(mask):
    key = mask.tobytes()
    if key not in _CACHE:
        plan, pats = _mask_plan(mask)
        nc = build_program(plan, pats.shape[0])
        _CACHE[key] = (nc, pats)
    return _CACHE[key]


def _chunk_major(xT):
    """[D, S] f32 -> [P, NQC*8*QC] f16 chunk-major for contiguous chunk DMA"""
    return np.ascontiguousarray(
        xT.astype(F16NP).reshape(8, P, NQC, QC).transpose(1, 2, 0, 3)
        .reshape(P, NQC * 8 * QC))


def make_in_maps(q, k, v, mask, wq, bq, wk, bk, wv, bv, wo, bo, pats):
    q, k, v = (np.asarray(a, np.float32) for a in (q, k, v))
    in_maps = []
    for c in range(NCORES):
        b, g = divmod(c, TP)
        sl = slice(g * EL, (g + 1) * EL)
        in_maps.append({
            "xq2": _chunk_major(q[b].T),
            "xk2": _chunk_major(k[b].T),
            "xv2": _chunk_major(v[b].T),
            "wqT": np.ascontiguousarray(wq[sl, :].T.astype(F16NP)),
            "wkT": np.ascontiguousarray(wk[sl, :].T.astype(F16NP)),
            "wvT": np.ascontiguousarray(wv[sl, :].T.astype(F16NP)),
            "woT": np.ascontiguousarray(wo[:, sl].T).astype(F16NP),
            "bq2": np.ascontiguousarray(bq[sl].reshape(2, P)),
            "pats": pats,
        })
    return in_maps


def assemble_output(results, bv, wo, bo):
    extra = (np.asarray(bv, np.float64) @ np.asarray(wo, np.float64).T
             + np.asarray(bo, np.float64)).astype(np.float32)  # [D]
    y = np.empty((B, S, D), np.float32)
    for b in range(B):
        acc = np.zeros((D, S), np.float32)
        for g in range(TP):
            yT = (results[b * TP + g]["yO"].astype(np.float32)
                  .reshape(NMT, NQC, P, QC).transpose(0, 2, 1, 3)
                  .reshape(D, S))
            acc += yT
        y[b] = acc.T + extra[None, :]
    return y


def kernel(q, k, v, mask, wq, bq, wk, bk, wv, bv, wo, bo):
    mask2d = np.asarray(mask).reshape(S, S)
    nc, pats = _get_program(mask2d)
    in_maps = make_in_maps(q, k, v, mask2d, wq, bq, wk, bk, wv, bv, wo, bo, pats)
    return assemble_output(run_spmd(nc, in_maps), bv, wo, bo)
